# revision 1
# baseline (speedup 1.0000x reference)
"""Deformable-attention transformer encoder layer on 8 Trainium2 cores.

Sharding: core = (batch b = core//2, L-half = core%2). Each core computes the
full value map for its batch element (needed for sampling) and runs the rest
of the layer on its 2688-query shard.

All activations are channel-major ([C partitions, seq free]); GEMMs run on
the tensor engine (MMDT selects exact float32 or fast float32r operands).
Deformable sampling: value maps are repacked per channel as
bf16 vertical pairs ((v[y,x], v[y+1,x]) in one 4-byte element) with a zero
border, so one GPSIMD indirect_copy index fetches all 4 bilinear taps of a
point; attention/bilinear/validity fold into 4 bf16 weights per point and the
weighted sum runs on DVE. Index/weight tiles reach the gather's interleaved
16-partition layout via a DRAM round-trip with strided access patterns.

Note: the fp32->int cast (floor) compensates for HW round-to-nearest; CoreSim
truncates instead, so CoreSim outputs diverge from hardware (hardware is the
reference: rel err ~1e-4 vs the fp32 oracle).
"""

import sys

for _p in ("/opt/trn_rl_repo",):
    if _p not in sys.path:
        sys.path.insert(0, _p)

import numpy as np
import concourse.bass as bass
import concourse.mybir as mybir
import concourse.tile as tile
from concourse import bacc
from concourse.bass import AP
from concourse.bass_utils import run_bass_kernel_spmd

F32 = mybir.dt.float32
F32R = mybir.dt.float32r
MMDT = mybir.dt.float32  # matmul operand dtype: float32 (exact) or float32r (fast)
BF16 = mybir.dt.bfloat16
I32 = mybir.dt.int32
U16 = mybir.dt.uint16
AF = mybir.ActivationFunctionType
OP = mybir.AluOpType
AX = mybir.AxisListType

B, L, C = 4, 5376, 256
NH, NL, NP = 8, 3, 4
FF = 2048
SHAPES = [(64, 64), (32, 32), (16, 16)]
LVLSTART = [0, 4096, 5120]
SHIFT = 16.0  # added to pixel coords so floor == int-trunc

# packed-map geometry: per level rows H+1 (y0 in [-1,H-1]), cols W+2 (x in [-1,W])
PK_BASE = []
_acc = 0
for _h, _w in SHAPES:
    PK_BASE.append(_acc)
    _acc += (_h + 1) * (_w + 2)
PKS = _acc + (_acc % 2) + 2  # even + safety pad


def _host_consts():
    W = np.zeros(96, np.float32)
    H = np.zeros(96, np.float32)
    WP2 = np.zeros(96, np.float32)
    KIDX = np.zeros(96, np.float32)
    for o in range(96):
        lvl = (o % 12) // 4
        h, w = SHAPES[lvl]
        W[o], H[o] = w, h
        WP2[o] = w + 2
        # level-relative: the gather uses a per-level slice of the packed map
        KIDX[o] = -(SHIFT - 1.0) * (w + 2) - (SHIFT - 1.0)
    c = {}
    c["CONSTS"] = np.stack(
        [
            np.full(96, SHIFT - 1.0, np.float32),  # 0: clamp lo for floor(coord)
            W + SHIFT - 1.0,                       # 1: clamp hi x
            H + SHIFT - 1.0,                       # 2: clamp hi y
            np.full(96, SHIFT - 1.0, np.float32),  # 3: mask lo
            W + SHIFT,                             # 4: mask hi x (strict)
            H + SHIFT,                             # 5: mask hi y (strict)
            WP2,                                   # 6
            KIDX,                                  # 7
        ],
        axis=1,
    ).astype(np.float32)
    ELX = np.zeros((3, 96), np.float32)
    ELY = np.zeros((3, 96), np.float32)
    for o in range(96):
        lvl = (o % 12) // 4
        ELX[lvl, o] = SHAPES[lvl][1]
        ELY[lvl, o] = SHAPES[lvl][0]
    c["E_LVLX"], c["E_LVLY"] = ELX, ELY
    ES = np.zeros((96, 8), np.float32)
    EE = np.zeros((8, 96), np.float32)
    for o in range(96):
        ES[o, o // 12] = 1.0
        EE[o // 12, o] = 1.0
    c["E_SUM"], c["E_EXP"] = ES, EE
    c["ONESC"] = np.ones((128, 1), np.float32)
    c["E_ONE1"] = np.ones((1, 128), np.float32)
    c["IDENT"] = np.eye(128, dtype=np.float32)
    return c


def _nchunks(n, step=512):
    out, i = [], 0
    while i < n:
        out.append((i, min(step, n - i)))
        i += step
    return out


def build_program(lq=L // 2, qc=448, gelu_hw=True):
    nc = bacc.Bacc("TRN2", target_bir_lowering=False, debug=False)
    consts = _host_consts()

    hd = {}
    names = []
    def dram_in(name, shape):
        hd[name] = nc.dram_tensor(name, list(shape), F32, kind="ExternalInput")
        names.append(name)
        return hd[name]

    for nm, shp in [
        ("src_full", (L, C)), ("src_own", (lq, C)), ("pos_own", (lq, C)),
        ("ref_own", (lq, NL, 2)),
        ("W_val", (C, C)), ("b_val", (C,)), ("W_off", (C, 192)), ("b_off", (192,)),
        ("W_attn", (C, 96)), ("b_attn", (96,)), ("W_out", (C, C)), ("b_out", (C,)),
        ("ln1_g", (C,)), ("ln1_b", (C,)), ("lin1_W", (C, FF)), ("lin1_b", (FF,)),
        ("lin2_W", (FF, C)), ("lin2_b", (C,)), ("ln2_g", (C,)), ("ln2_b", (C,)),
    ]:
        dram_in(nm, shp)
    for k, v in consts.items():
        dram_in(k, v.shape)
    y_own = nc.dram_tensor("y_own", [lq, C], F32, kind="ExternalOutput")
    idx_dram = nc.dram_tensor("idx_scratch", [96 * lq], U16, kind="Internal")
    wp_dram = nc.dram_tensor("wp_scratch", [96 * lq * 2], F32, kind="Internal")

    with tile.TileContext(nc) as tc:
        with nc.allow_low_precision(reason="float32r rounding of matmul operands"):
            _body(tc, nc, hd, y_own, idx_dram, wp_dram, lq, qc, gelu_hw)
    nc.compile()
    return nc, names, consts


def _load_T(nc, pool, ps, ident, dram_ap, rows, dsts, tag):
    """Load a [rows, 256] DRAM region transposed into two [128, rows] tiles
    (channel halves) via straight DMA + PE transpose."""
    assert rows % 128 == 0
    for ch in range(rows // 128):
        o = ch * 128
        t = pool.tile([128, 256], F32, tag=f"{tag}_ld", name=f"{tag}_ld", bufs=3)
        nc.sync.dma_start(t[:], dram_ap[o : o + 128, :])
        for m, dst in enumerate(dsts):
            pst = ps.tile([128, 128], F32, tag=f"{tag}_ps", name=f"{tag}_ps", bufs=2)
            nc.tensor.transpose(pst[:], t[:, 128 * m : 128 * m + 128], ident[:])
            nc.scalar.activation(dst[:, o : o + 128], pst[:], AF.Copy)


def _body(tc, nc, d, y_own, idx_dram, wp_dram, lq, qc, gelu_hw):
    ACT = nc.scalar
    DVE = nc.vector
    GPS = nc.gpsimd
    NQC = _nchunks(lq, 512)
    NSC = lq // qc
    assert lq % qc == 0 and qc % 16 == 0

    def ap(nm):
        return d[nm].ap()

    st = tc.alloc_tile_pool(name="wpool", bufs=1)

    def load(nm_or_ap, p, f, tag, dt=F32):
        src = ap(nm_or_ap) if isinstance(nm_or_ap, str) else nm_or_ap
        t = st.tile([p, f], dt, tag=tag, name=tag)
        if dt == MMDT and src.dtype != MMDT:
            src = src.bitcast(MMDT)
        nc.sync.dma_start(t[:, :], src)
        return t

    # ---- persistent weights/consts ----
    wval = [[load(ap("W_val")[128 * k : 128 * k + 128, 128 * m : 128 * m + 128], 128, 128, f"wval{k}{m}", dt=MMDT) for m in range(2)] for k in range(2)]
    woffx = [load(AP(d["W_off"], 128 * k * 192, [[192, 128], [2, 96]]), 128, 96, f"woffx{k}", dt=MMDT) for k in range(2)]
    woffy = [load(AP(d["W_off"], 128 * k * 192 + 1, [[192, 128], [2, 96]]), 128, 96, f"woffy{k}", dt=MMDT) for k in range(2)]
    wattn = [load(ap("W_attn")[128 * k : 128 * k + 128, :], 128, 96, f"wattn{k}", dt=MMDT) for k in range(2)]
    wout = [[load(ap("W_out")[128 * k : 128 * k + 128, 128 * m : 128 * m + 128], 128, 128, f"wout{k}{m}", dt=MMDT) for m in range(2)] for k in range(2)]
    lin1 = [load(ap("lin1_W")[128 * k : 128 * k + 128, :], 128, FF, f"lin1{k}", dt=MMDT) for k in range(2)]
    lin2 = load(AP(d["lin2_W"], 0, [[256, 128], [128 * 256, 16], [1, 256]]), 128, 16 * 256, "lin2", dt=MMDT)
    elx = load("E_LVLX", 3, 96, "elx", dt=MMDT)
    ely = load("E_LVLY", 3, 96, "ely", dt=MMDT)
    esum = load("E_SUM", 96, 8, "esum", dt=MMDT)
    eexp = load("E_EXP", 8, 96, "eexp", dt=MMDT)
    onesc = load("ONESC", 128, 1, "onesc", dt=MMDT)
    eone1 = load("E_ONE1", 1, 128, "eone1", dt=MMDT)
    ident = load("IDENT", 128, 128, "ident")
    cst = load("CONSTS", 96, 8, "cst")
    bvec = lambda nm, m, tag: load(AP(d[nm], 128 * m, [[1, 128], [1, 1]]), 128, 1, tag)
    bval = [bvec("b_val", m, f"bval{m}") for m in range(2)]
    bout = [bvec("b_out", m, f"bout{m}") for m in range(2)]
    l1g = [bvec("ln1_g", m, f"l1g{m}") for m in range(2)]
    l1b = [bvec("ln1_b", m, f"l1b{m}") for m in range(2)]
    l2g = [bvec("ln2_g", m, f"l2g{m}") for m in range(2)]
    l2b = [bvec("ln2_b", m, f"l2b{m}") for m in range(2)]
    lin2b = [bvec("lin2_b", m, f"lin2b{m}") for m in range(2)]
    battn = load(AP(d["b_attn"], 0, [[1, 96], [1, 1]]), 96, 1, "battn")
    lin1b = load(AP(d["lin1_b"], 0, [[1, 128], [128, 16]]), 128, 16, "lin1b")
    boffx_r = load(AP(d["b_off"], 0, [[2, 96], [1, 1]]), 96, 1, "boffxr")
    boffy_r = load(AP(d["b_off"], 1, [[2, 96], [1, 1]]), 96, 1, "boffyr")
    boffx = st.tile([96, 1], F32, tag="boffx", name="boffx")
    boffy = st.tile([96, 1], F32, tag="boffy", name="boffy")
    DVE.tensor_scalar_add(boffx[:], boffx_r[:], SHIFT - 0.5)
    DVE.tensor_scalar_add(boffy[:], boffy_r[:], SHIFT - 0.5)

    # ================= phase 1: value GEMM + q =================
    p_pk = tc.alloc_tile_pool(name="ppk", bufs=1)
    pk = [p_pk.tile([128, PKS], F32, tag=f"pk{m}", name=f"pk{m}") for m in range(2)]
    p_q = tc.alloc_tile_pool(name="pq", bufs=1)
    qT = [p_q.tile([128, lq], F32, tag=f"qT{m}", name=f"qT{m}") for m in range(2)]
    p_val = tc.alloc_tile_pool(name="pval", bufs=1)
    valT = [p_val.tile([128, L], F32, tag=f"valT{m}", name=f"valT{m}") for m in range(2)]

    pbig = tc.alloc_tile_pool(name="pbig", bufs=1)
    ps1 = tc.alloc_tile_pool(name="ps1", bufs=2, space="PSUM")
    # qT = (src_own + pos_own) transposed, fused at load time
    for ch in range(lq // 128):
        o = ch * 128
        ta = pbig.tile([128, 256], F32, tag="q_lda", name="q_lda", bufs=3)
        tb = pbig.tile([128, 256], F32, tag="q_ldb", name="q_ldb", bufs=3)
        nc.sync.dma_start(ta[:], ap("src_own")[o : o + 128, :])
        nc.sync.dma_start(tb[:], ap("pos_own")[o : o + 128, :])
        tq = pbig.tile([128, 256], F32, tag="q_sum", name="q_sum", bufs=3)
        DVE.tensor_copy(tq[:], ta[:])
        DVE.tensor_tensor(tq[:], tq[:], tb[:], OP.add)
        for m in range(2):
            pst = ps1.tile([128, 128], F32, tag="q_ps", name="q_ps", bufs=2)
            nc.tensor.transpose(pst[:], tq[:, 128 * m : 128 * m + 128], ident[:])
            ACT.activation(qT[m][:, o : o + 128].bitcast(MMDT), pst[:], AF.Copy)
    # value GEMM with chunked transposed loads of src_full
    for (o, n) in _nchunks(L, 512):
        srcc = [pbig.tile([128, 512], F32, tag=f"srcc{k}", name=f"srcc{k}", bufs=2) for k in range(2)]
        for sub in range(n // 128):
            tl = pbig.tile([128, 256], F32, tag="v_ld", name="v_ld", bufs=3)
            nc.sync.dma_start(tl[:], ap("src_full")[o + 128 * sub : o + 128 * sub + 128, :])
            for k in range(2):
                pst = ps1.tile([128, 128], F32, tag="v_ps", name="v_ps", bufs=2)
                nc.tensor.transpose(pst[:], tl[:, 128 * k : 128 * k + 128], ident[:])
                ACT.activation(srcc[k][:, 128 * sub : 128 * sub + 128].bitcast(MMDT), pst[:], AF.Copy)
        for m in range(2):
            ps = ps1.tile([128, 512], F32, tag="ps_val", name="ps_val")
            for k in range(2):
                nc.tensor.matmul(ps[:, :n], wval[k][m][:].bitcast(MMDT), srcc[k][:, :n].bitcast(MMDT), start=(k == 0), stop=(k == 1))
            ACT.activation(valT[m][:, o : o + n], ps[:, :n], AF.Identity, bias=bval[m][:])
    ps1.release()
    pbig.release()

    # ---- packed value maps (from valT) ----
    for m in range(2):
        GPS.memset(pk[m][:], 0.0)
        pb = pk[m][:].bitcast(BF16)  # [128, 2*PKS]
        for lvl in range(3):
            H, W = SHAPES[lvl]
            base2 = PK_BASE[lvl] * 2
            rs2 = (W + 2) * 2
            src = valT[m][:, LVLSTART[lvl] : LVLSTART[lvl] + H * W].rearrange("p (y x) -> p y x", x=W)
            v0 = pb[:, base2 + rs2 : base2 + rs2 + H * rs2].rearrange(
                "p (y c two) -> p y c two", c=W + 2, two=2)[:, :, 1 : W + 1, 0]
            v1 = pb[:, base2 : base2 + H * rs2].rearrange(
                "p (y c two) -> p y c two", c=W + 2, two=2)[:, :, 1 : W + 1, 1]
            DVE.tensor_copy(v0, src)
            DVE.tensor_copy(v1, src)
    p_val.release()

    # ========= phase 2: chunked offset/attn math -> idx/weight DRAM =========
    pom = tc.alloc_tile_pool(name="pom", bufs=2)
    ps2 = tc.alloc_tile_pool(name="ps2", bufs=1, space="PSUM")
    for (o, n) in NQC:
        ps_x = ps2.tile([96, 512], F32, tag="ps_x", name="ps_x")
        ps_y = ps2.tile([96, 512], F32, tag="ps_y", name="ps_y")
        ps_a = ps2.tile([96, 512], F32, tag="ps_a", name="ps_a")
        for k in range(2):
            nc.tensor.matmul(ps_x[:, :n], woffx[k][:].bitcast(MMDT), qT[k][:, o : o + n].bitcast(MMDT), start=(k == 0), stop=False)
        refx = pom.tile([3, 512], MMDT, tag="refx", name="refx")
        refy = pom.tile([3, 512], MMDT, tag="refy", name="refy")
        nc.sync.dma_start(refx[:, :n], AP(d["ref_own"], 6 * o, [[2, 3], [6, n]]).bitcast(MMDT))
        nc.sync.dma_start(refy[:, :n], AP(d["ref_own"], 6 * o + 1, [[2, 3], [6, n]]).bitcast(MMDT))
        nc.tensor.matmul(ps_x[:, :n], elx[:].bitcast(MMDT), refx[:, :n].bitcast(MMDT), start=False, stop=True)
        for k in range(2):
            nc.tensor.matmul(ps_y[:, :n], woffy[k][:].bitcast(MMDT), qT[k][:, o : o + n].bitcast(MMDT), start=(k == 0), stop=False)
        nc.tensor.matmul(ps_y[:, :n], ely[:].bitcast(MMDT), refy[:, :n].bitcast(MMDT), start=False, stop=True)
        for k in range(2):
            nc.tensor.matmul(ps_a[:, :n], wattn[k][:].bitcast(MMDT), qT[k][:, o : o + n].bitcast(MMDT), start=(k == 0), stop=(k == 1))

        xs = pom.tile([96, 512], F32, tag="xs", name="xs")
        ys = pom.tile([96, 512], F32, tag="ys", name="ys")
        ex = pom.tile([96, 512], F32, tag="ex", name="ex")
        ACT.activation(xs[:, :n], ps_x[:, :n], AF.Identity, bias=boffx[:])
        ACT.activation(ys[:, :n], ps_y[:, :n], AF.Identity, bias=boffy[:])
        ACT.activation(ex[:, :n].bitcast(MMDT), ps_a[:, :n], AF.Exp, bias=battn[:])

        ps_d = ps2.tile([8, 512], F32, tag="ps_d", name="ps_d")
        nc.tensor.matmul(ps_d[:, :n], esum[:].bitcast(MMDT), ex[:, :n].bitcast(MMDT), start=True, stop=True)
        rec = pom.tile([8, 512], F32, tag="rec", name="rec")
        DVE.reciprocal(rec[:, :n].bitcast(MMDT), ps_d[:, :n])
        ps_r = ps2.tile([96, 512], F32, tag="ps_r", name="ps_r")
        nc.tensor.matmul(ps_r[:, :n], eexp[:].bitcast(MMDT), rec[:, :n].bitcast(MMDT), start=True, stop=True)
        am = pom.tile([96, 512], F32, tag="am", name="am")
        DVE.tensor_tensor(am[:, :n], ex[:, :n], ps_r[:, :n], OP.mult)

        ti = pom.tile([96, 512], I32, tag="ti", name="ti")
        fx = pom.tile([96, 512], F32, tag="fx", name="fx")
        fy = pom.tile([96, 512], F32, tag="fy", name="fy")
        wx = pom.tile([96, 512], F32, tag="wx", name="wx")
        wy = pom.tile([96, 512], F32, tag="wy", name="wy")
        tmp = pom.tile([96, 512], F32, tag="tmp", name="tmp")
        for (coord, fl, wf, hi_clamp, hi_mask) in (
            (xs, fx, wx, 1, 4),
            (ys, fy, wy, 2, 5),
        ):
            # HW fp32->int32 cast rounds to nearest; floor(v) == rint(v - 0.5)
            # (coords are never exactly half-integers in practice; bilinear is
            # continuous across the frac boundary so eps cases are benign)
            DVE.tensor_scalar_add(tmp[:, :n], coord[:, :n], -0.5)
            DVE.tensor_copy(ti[:, :n], tmp[:, :n])
            DVE.tensor_copy(fl[:, :n], ti[:, :n])
            DVE.tensor_tensor(wf[:, :n], coord[:, :n], fl[:, :n], OP.subtract)
            DVE.tensor_scalar(tmp[:, :n], coord[:, :n], cst[:, 3:4], None, OP.is_ge)
            DVE.tensor_tensor(am[:, :n], am[:, :n], tmp[:, :n], OP.mult)
            DVE.tensor_scalar(tmp[:, :n], coord[:, :n], cst[:, hi_mask : hi_mask + 1], None, OP.is_lt)
            DVE.tensor_tensor(am[:, :n], am[:, :n], tmp[:, :n], OP.mult)
            DVE.tensor_scalar(fl[:, :n], fl[:, :n], cst[:, 0:1], None, OP.max)
            DVE.tensor_scalar(fl[:, :n], fl[:, :n], cst[:, hi_clamp : hi_clamp + 1], None, OP.min)
        ACT.activation(tmp[:, :n], fy[:, :n], AF.Identity, bias=cst[:, 7:8], scale=cst[:, 6:7])
        DVE.tensor_tensor(tmp[:, :n], tmp[:, :n], fx[:, :n], OP.add)
        # u16 cast with q_lo-swizzled free order: iu[o, q_lo*(n/16) + q_hi]
        iu = pom.tile([96, 512], U16, tag="iu", name="iu")
        nqh = n // 16
        iu_v = iu[:, :n].rearrange("p (ql qh) -> p ql qh", ql=16)
        tmp_v = tmp[:, :n].rearrange("p (qh ql) -> p ql qh", ql=16)
        DVE.tensor_copy(iu_v, tmp_v)
        # dram layout: row o, position q_lo*(lq/16) + q_hi_global
        dst = AP(idx_dram, o // 16, [[lq, 96], [lq // 16, 16], [1, nqh]])
        nc.sync.dma_start(dst, iu_v)

        wy1 = pom.tile([96, 512], F32, tag="wy1", name="wy1")
        wx1 = pom.tile([96, 512], F32, tag="wx1", name="wx1")
        ACT.activation(wy1[:, :n], wy[:, :n], AF.Identity, bias=1.0, scale=-1.0)
        ACT.activation(wx1[:, :n], wx[:, :n], AF.Identity, bias=1.0, scale=-1.0)
        atop = pom.tile([96, 512], F32, tag="atop", name="atop")
        abot = pom.tile([96, 512], F32, tag="abot", name="abot")
        DVE.tensor_tensor(atop[:, :n], am[:, :n], wy1[:, :n], OP.mult)
        DVE.tensor_tensor(abot[:, :n], am[:, :n], wy[:, :n], OP.mult)
        wp = pom.tile([96, 512 * 2], F32, tag="wp", name="wp")
        wpb = wp[:].bitcast(BF16).rearrange("p (q four) -> p q four", four=4)
        DVE.tensor_tensor(wpb[:, :n, 0], atop[:, :n], wx1[:, :n], OP.mult)
        DVE.tensor_tensor(wpb[:, :n, 1], abot[:, :n], wx1[:, :n], OP.mult)
        DVE.tensor_tensor(wpb[:, :n, 2], atop[:, :n], wx[:, :n], OP.mult)
        DVE.tensor_tensor(wpb[:, :n, 3], abot[:, :n], wx[:, :n], OP.mult)
        nc.sync.dma_start(AP(wp_dram, 0, [[2 * lq, 96], [1, 2 * lq]])[:, 2 * o : 2 * (o + n)], wp[:, : 2 * n])
    ps2.release()
    pom.release()
    p_q.release()

    # ================= phase 4: sampling =================
    p_x1 = tc.alloc_tile_pool(name="px1", bufs=1)
    x1 = [p_x1.tile([128, lq], F32, tag=f"x1{m}", name=f"x1{m}") for m in range(2)]
    p_out = tc.alloc_tile_pool(name="pout", bufs=1)
    outT = [p_out.tile([128, lq], F32, tag=f"outT{m}", name=f"outT{m}") for m in range(2)]
    QH = lq // 16          # global q_hi count
    QRUN = qc              # queries per gather call (multiple of 16, divides lq)
    FRUN = QRUN // 16      # idx-tile columns per call
    psmp = tc.alloc_tile_pool(name="psamp", bufs=2)
    LVLSZ = [(h + 1) * (w + 2) for h, w in SHAPES]
    for t in range(2):
        pk_lv = [
            pk[t][:, PK_BASE[l] : PK_BASE[l] + LVLSZ[l]].rearrange("p (a two) -> p a two", two=2)
            for l in range(3)
        ]
        idx16 = psmp.tile([128, 12 * QH], U16, tag="idx16", name="idx16", bufs=1)
        for g in range(8):
            h = 4 * t + g // 2
            src_ap = AP(idx_dram, (12 * h) * lq, [[QH, 16], [lq, 12], [1, QH]])
            dst_ap = idx16[16 * g : 16 * g + 16, :].rearrange("p (lp qh) -> p lp qh", lp=12)
            nc.sync.dma_start(dst_ap, src_ap)
        for lp in range(12):
            for cq in range(lq // QRUN):
                q0 = cq * QRUN
                wt = psmp.tile([128, QRUN * 2], F32, tag="wt", name="wt")
                for j in range(4):
                    h = 4 * t + j
                    src_ap = AP(wp_dram, (12 * h + lp) * (lq * 2) + 2 * q0, [[0, 32], [1, QRUN * 2]])
                    nc.sync.dma_start(wt[32 * j : 32 * j + 32, :], src_ap)
                g_t = psmp.tile([128, QRUN, 2], F32, tag="gt", name="gt")
                GPS.indirect_copy(g_t[:], pk_lv[lp // 4], idx16[:, lp * QH + q0 // 16 : lp * QH + q0 // 16 + FRUN], i_know_ap_gather_is_preferred=True)
                prod = psmp.tile([128, QRUN * 4], BF16, tag="prod", name="prod")
                DVE.tensor_tensor(prod[:], g_t[:].bitcast(BF16).rearrange("p a b -> p (a b)"), wt[:].bitcast(BF16), OP.mult)
                pv = prod[:].rearrange("p (i e y) -> p i e y", e=2, y=2)
                acc = psmp.tile([128, QRUN, 2], BF16, tag="acc", name="acc")
                DVE.tensor_tensor(acc[:], pv[:, :, 0, :], pv[:, :, 1, :], OP.add)
                red = psmp.tile([128, QRUN], F32, tag="red", name="red")
                DVE.tensor_reduce(red[:], acc[:], axis=AX.X, op=OP.add)
                if lp == 0:
                    DVE.tensor_copy(outT[t][:, q0 : q0 + QRUN].bitcast(MMDT), red[:])
                else:
                    DVE.tensor_tensor(outT[t][:, q0 : q0 + QRUN].bitcast(MMDT), outT[t][:, q0 : q0 + QRUN], red[:], OP.add)
    psmp.release()

    # ================= phase 5: out-proj + LN1 =================
    pres = tc.alloc_tile_pool(name="pres", bufs=1)
    ps5 = tc.alloc_tile_pool(name="ps5", bufs=2, space="PSUM")
    srcO2 = [pres.tile([128, lq], F32, tag=f"srcO2{m}", name=f"srcO2{m}") for m in range(2)]
    _load_T(nc, pres, ps5, ident, ap("src_own"), lq, srcO2, "srcO2")
    ps5.release()
    _lnorm(tc, nc, pres, lq, x1, gemm_k=outT, gemm_w=wout, gemm_b=bout,
           resid=srcO2, g=l1g, bb=l1b, onesc=onesc, eone1=eone1)
    pres.release()
    p_out.release()

    # ================= phase 6: FFN + LN2 -> y (in place on x1) =================
    x2 = x1
    pffn = tc.alloc_tile_pool(name="pffn", bufs=2)
    ps6 = tc.alloc_tile_pool(name="ps6", bufs=2, space="PSUM")
    ps6b = tc.alloc_tile_pool(name="ps6b", bufs=1, space="PSUM")
    for (o, n) in NQC:
        ps_h2 = [ps6b.tile([128, 512], F32, tag=f"ps_h2{m}", name=f"ps_h2{m}") for m in range(2)]
        for mf in range(16):
            ps_h1 = ps6.tile([128, 512], F32, tag="ps_h1", name="ps_h1")
            for k in range(2):
                nc.tensor.matmul(ps_h1[:, :n], lin1[k][:, 128 * mf : 128 * mf + 128].bitcast(MMDT), x1[k][:, o : o + n].bitcast(MMDT), start=(k == 0), stop=(k == 1))
            h1 = pffn.tile([128, 512], F32, tag="h1", name="h1")
            ACT.activation(h1[:, :n].bitcast(MMDT), ps_h1[:, :n], AF.Gelu if gelu_hw else AF.Tanh, bias=lin1b[:, mf : mf + 1])
            for m in range(2):
                nc.tensor.matmul(ps_h2[m][:, :n], lin2[:, 256 * mf + 128 * m : 256 * mf + 128 * m + 128].bitcast(MMDT), h1[:, :n].bitcast(MMDT), start=(mf == 0), stop=(mf == 15))
        for m in range(2):
            h2s = pffn.tile([128, 512], F32, tag=f"h2s{m}", name=f"h2s{m}")
            ACT.activation(h2s[:, :n], ps_h2[m][:, :n], AF.Identity, bias=lin2b[m][:])
            DVE.tensor_tensor(x2[m][:, o : o + n].bitcast(MMDT), h2s[:, :n], x1[m][:, o : o + n], OP.add)
    ps6b.release()
    ps6.release()
    _lnorm(tc, nc, pffn, lq, x2, gemm_k=None, gemm_w=None, gemm_b=None,
           resid=None, g=l2g, bb=l2b, onesc=onesc, eone1=eone1, inplace=x2)
    pffn.release()

    # ---- transpose out ----
    pyo = tc.alloc_tile_pool(name="pyo", bufs=3)
    psT = tc.alloc_tile_pool(name="psT", bufs=2, space="PSUM")
    for ch in range(lq // 128):
        o = ch * 128
        yq = pyo.tile([128, 256], F32, tag="yq", name="yq")
        for m in range(2):
            ps_t = psT.tile([128, 128], F32, tag="ps_t", name="ps_t")
            nc.tensor.transpose(ps_t[:], x2[m][:, o : o + 128], ident[:])
            ACT.activation(yq[:, 128 * m : 128 * m + 128], ps_t[:], AF.Copy)
        nc.sync.dma_start(y_own.ap()[o : o + 128, :], yq[:])
    psT.release()
    pyo.release()
    p_x1.release()
    p_pk.release()
    st.release()


def _lnorm(tc, nc, pool, lq, dst, gemm_k, gemm_w, gemm_b, resid, g, bb,
           onesc, eone1, inplace=None, dst_dt=None):
    """dst[m] = LN(x)*g + b over channels (2 x 128 partitions).

    x = resid + (W.T @ gemm_k + gemm_b) when gemm_w given, else x = inplace.
    """
    ACT, DVE = nc.scalar, nc.vector
    psl = tc.alloc_tile_pool(name="psln", bufs=1, space="PSUM")
    eps = pool.tile([1, 1], F32, tag="ln_eps", name="ln_eps")
    nc.any.memset(eps[:], 1e-5)
    for (o, n) in _nchunks(lq, 512):
        xs = []
        if gemm_w is not None:
            for m in range(2):
                ps = psl.tile([128, 512], F32, tag=f"ln_ps{m}", name=f"ln_ps{m}")
                for k in range(2):
                    nc.tensor.matmul(ps[:, :n], gemm_w[k][m][:].bitcast(MMDT), gemm_k[k][:, o : o + n].bitcast(MMDT), start=(k == 0), stop=(k == 1))
                x = pool.tile([128, 512], F32, tag=f"ln_x{m}", name=f"ln_x{m}")
                ACT.activation(x[:, :n].bitcast(MMDT), ps[:, :n], AF.Identity, bias=gemm_b[m][:])
                DVE.tensor_tensor(x[:, :n].bitcast(MMDT), x[:, :n], resid[m][:, o : o + n], OP.add)
                xs.append(x[:, :n])
        else:
            xs = [inplace[m][:, o : o + n] for m in range(2)]

        ps_s = psl.tile([1, 512], F32, tag="ln_s1", name="ln_s1")
        ps_s2 = psl.tile([1, 512], F32, tag="ln_s2", name="ln_s2")
        sq = pool.tile([128, 512], F32, tag="ln_sq", name="ln_sq")
        for m in range(2):
            nc.tensor.matmul(ps_s[:, :n], onesc[:].bitcast(MMDT), xs[m].bitcast(MMDT), start=(m == 0), stop=(m == 1))
        for m in range(2):
            ACT.activation(sq[:, :n].bitcast(MMDT), xs[m], AF.Square)
            nc.tensor.matmul(ps_s2[:, :n], onesc[:].bitcast(MMDT), sq[:, :n].bitcast(MMDT), start=(m == 0), stop=(m == 1))
        mean = pool.tile([1, 512], F32, tag="ln_mean", name="ln_mean")
        ACT.activation(mean[:, :n].bitcast(MMDT), ps_s[:, :n], AF.Copy, scale=1.0 / 256.0)
        ms = pool.tile([1, 512], F32, tag="ln_ms", name="ln_ms")
        DVE.tensor_tensor(ms[:, :n], mean[:, :n], mean[:, :n], OP.mult)
        var = pool.tile([1, 512], F32, tag="ln_var", name="ln_var")
        ACT.activation(var[:, :n], ps_s2[:, :n], AF.Copy, scale=1.0 / 256.0)
        DVE.tensor_tensor(var[:, :n], var[:, :n], ms[:, :n], OP.subtract)
        std = pool.tile([1, 512], F32, tag="ln_std", name="ln_std")
        ACT.activation(std[:, :n], var[:, :n], AF.Sqrt, bias=eps[:])
        rstd = pool.tile([1, 512], F32, tag="ln_rstd", name="ln_rstd")
        DVE.reciprocal(rstd[:, :n].bitcast(MMDT), std[:, :n])
        ps_bm = psl.tile([128, 512], F32, tag="ln_bm", name="ln_bm")
        ps_br = psl.tile([128, 512], F32, tag="ln_br", name="ln_br")
        nc.tensor.matmul(ps_bm[:, :n], eone1[:].bitcast(MMDT), mean[:, :n].bitcast(MMDT), start=True, stop=True)
        nc.tensor.matmul(ps_br[:, :n], eone1[:].bitcast(MMDT), rstd[:, :n].bitcast(MMDT), start=True, stop=True)
        for m in range(2):
            t = pool.tile([128, 512], F32, tag="ln_t", name="ln_t")
            DVE.tensor_tensor(t[:, :n], xs[m], ps_bm[:, :n], OP.subtract)
            DVE.tensor_tensor(t[:, :n], t[:, :n], ps_br[:, :n], OP.mult)
            ACT.activation(dst[m][:, o : o + n].bitcast(dst_dt or MMDT), t[:, :n], AF.Identity, bias=bb[m][:], scale=g[m][:])
    psl.release()


# ======================= host side =======================

_CACHE = {}


def _get_program():
    if "nc" not in _CACHE:
        nc, names, consts = build_program()
        _CACHE["nc"] = nc
        _CACHE["names"] = names
        _CACHE["consts"] = consts
    return _CACHE["nc"], _CACHE["names"], _CACHE["consts"]


def make_in_maps(inputs, lq=L // 2, consts=None):
    consts = consts if consts is not None else _host_consts()
    src = np.ascontiguousarray(np.asarray(inputs["src"], np.float32))
    pos = np.ascontiguousarray(np.asarray(inputs["pos"], np.float32))
    ref = np.ascontiguousarray(np.asarray(inputs["reference_points"], np.float32))
    weights = {
        k: np.ascontiguousarray(np.asarray(inputs[k]), np.float32)
        for k in ["W_val", "b_val", "W_off", "b_off", "W_attn", "b_attn",
                   "W_out", "b_out", "ln1_g", "ln1_b", "lin1_W", "lin1_b",
                   "lin2_W", "lin2_b", "ln2_g", "ln2_b"]
    }
    in_maps = []
    for core in range(8):
        b, half = core // 2, core % 2
        o = half * (L // 2)
        m = {
            "src_full": src[b],
            "src_own": np.ascontiguousarray(src[b, o : o + lq]),
            "pos_own": np.ascontiguousarray(pos[b, o : o + lq]),
            "ref_own": np.ascontiguousarray(ref[b, o : o + lq]),
        }
        m.update(weights)
        m.update(consts)
        in_maps.append(m)
    return in_maps


def kernel(**inputs):
    nc, names, consts = _get_program()
    in_maps = make_in_maps(inputs, consts=consts)
    res = run_bass_kernel_spmd(nc, in_maps, core_ids=list(range(8)))
    lq = L // 2
    out = np.zeros((B, L, C), np.float32)
    for core in range(8):
        b, half = core // 2, core % 2
        o = half * lq
        out[b, o : o + lq] = res.results[core]["y_own"]
    return out



# revision 9
# speedup vs baseline: 1.7902x; 1.7902x over previous
"""Deformable-attention transformer encoder layer on 8 Trainium2 cores.

Sharding: core = (batch b = core//2, L-half = core%2). Each core computes the
full value map for its batch element (needed for sampling) and runs the rest
of the layer on its 2688-query shard.

All activations are channel-major ([C partitions, seq free]); GEMMs run on
the tensor engine with float32r operands (1 cyc/row at N>=256).
The 96 (head,level,point) rows are laid out as (level,point,head) so the
per-head sampling weights for one (level,point) sit on 4 adjacent rows and
reach the gather's replicated [4x32, q] layout with a single broadcast DMA.
Deformable sampling: value maps are repacked per channel as bf16 vertical
pairs ((v[y,x], v[y+1,x]) in one 4-byte element) with a zero border, so one
GPSIMD indirect_copy index fetches all 4 bilinear taps of a point; gathers run
full-length (one per (half, level-point)). The 4 bf16 tap weights fold
attention/bilinear/validity; DVE applies them in one in-place bf16 multiply
and the tap/point accumulation runs on the tensor engine as bf16
identity-matmuls into PSUM.

Note: the fp32->int cast (floor) compensates for HW round-to-nearest; CoreSim
truncates instead, so CoreSim outputs diverge from hardware (hardware is the
reference vs the fp32 oracle).
"""

import sys

for _p in ("/opt/trn_rl_repo",):
    if _p not in sys.path:
        sys.path.insert(0, _p)

import numpy as np
import concourse.bass as bass
import concourse.mybir as mybir
import concourse.tile as tile
from concourse import bacc
from concourse.bass import AP
from concourse.bass_utils import run_bass_kernel_spmd

F32 = mybir.dt.float32
MMDT = mybir.dt.float32r  # matmul operand dtype (bit-identical to f32)
BF16 = mybir.dt.bfloat16
I32 = mybir.dt.int32
U16 = mybir.dt.uint16
AF = mybir.ActivationFunctionType
OP = mybir.AluOpType
AX = mybir.AxisListType

B, L, C = 4, 5376, 256
NH, NL, NP = 8, 3, 4
FF = 2048
SHAPES = [(64, 64), (32, 32), (16, 16)]
LVLSTART = [0, 4096, 5120]
SHIFT = 16.0  # added to pixel coords so floor == int-trunc
LQ = L // 2
QH = LQ // 16  # 168

# packed-map geometry: per level rows H+1 (y0 in [-1,H-1]), cols W+2 (x in [-1,W])
PK_BASE = []
_acc = 0
for _h, _w in SHAPES:
    PK_BASE.append(_acc)
    _acc += (_h + 1) * (_w + 2)
PKS = _acc + (_acc % 2) + 2  # even + safety pad
LVLSZ = [(h + 1) * (w + 2) for h, w in SHAPES]

# value-GEMM chunks, level aligned (tokens o..o+n inside one level)
VCHUNKS = (
    [(o, 512, 0) for o in range(0, 4096, 512)]
    + [(o, 512, 1) for o in range(4096, 5120, 512)]
    + [(5120, 256, 2)]
)


def _row_lph(r):
    # partition r -> (level, point, head) with r = (l*4+p)*8 + h
    return r // 32, (r // 8) % 4, r % 8


def _host_consts():
    W = np.zeros(96, np.float32)
    H = np.zeros(96, np.float32)
    WP2 = np.zeros(96, np.float32)
    KIDX = np.zeros(96, np.float32)
    for r in range(96):
        lvl, _, _ = _row_lph(r)
        h, w = SHAPES[lvl]
        W[r], H[r] = w, h
        WP2[r] = w + 2
        # level-relative: the gather uses a per-level slice of the packed map
        KIDX[r] = -(SHIFT - 1.0) * (w + 2) - (SHIFT - 1.0)
    c = {}
    c["CONSTS"] = np.stack(
        [
            np.full(96, SHIFT - 1.0, np.float32),  # 0: clamp lo for floor
            W + SHIFT - 1.0,                       # 1: clamp hi x
            H + SHIFT - 1.0,                       # 2: clamp hi y
            np.full(96, SHIFT - 1.5, np.float32),  # 3: mask lo (on cs)
            W + SHIFT - 0.5,                       # 4: mask hi x (on cs)
            H + SHIFT - 0.5,                       # 5: mask hi y (on cs)
            WP2,                                   # 6
            KIDX,                                  # 7
        ],
        axis=1,
    ).astype(np.float32)
    ELX = np.zeros((3, 96), np.float32)
    ELY = np.zeros((3, 96), np.float32)
    for r in range(96):
        lvl, _, _ = _row_lph(r)
        ELX[lvl, r] = SHAPES[lvl][1]
        ELY[lvl, r] = SHAPES[lvl][0]
    c["E_LVLX"], c["E_LVLY"] = ELX, ELY
    ES = np.zeros((96, 8), np.float32)
    EE = np.zeros((8, 96), np.float32)
    for r in range(96):
        ES[r, r % 8] = 1.0
        EE[r % 8, r] = 1.0
    c["E_SUM"], c["E_EXP"] = ES, EE
    c["ONESC"] = np.ones((128, 1), np.float32)
    c["E_ONE1"] = np.ones((1, 128), np.float32)
    c["IDENT"] = np.eye(128, dtype=np.float32)
    return c


def _perm96():
    # perm[r] = original (h,l,p) index for new row r = (l*4+p)*8 + h
    perm = np.zeros(96, np.int64)
    for r in range(96):
        lvl, p, h = _row_lph(r)
        perm[r] = h * 12 + lvl * 4 + p
    return perm


def _nchunks(n, step):
    out, i = [], 0
    while i < n:
        out.append((i, min(step, n - i)))
        i += step
    return out


def build_program(lq=LQ, qc=448, gelu_hw=True):
    nc = bacc.Bacc("TRN2", target_bir_lowering=False, debug=False)
    consts = _host_consts()

    hd = {}
    names = []
    def dram_in(name, shape):
        hd[name] = nc.dram_tensor(name, list(shape), F32, kind="ExternalInput")
        names.append(name)
        return hd[name]

    for nm, shp in [
        ("src_full", (L, C)), ("src_own", (lq, C)), ("pos_own", (lq, C)),
        ("ref_own", (lq, NL, 2)),
        ("W_val", (C, C)), ("b_val", (C,)), ("W_off", (C, 192)), ("b_off", (192,)),
        ("W_attn", (C, 96)), ("b_attn", (96,)), ("W_out", (C, C)), ("b_out", (C,)),
        ("ln1_g", (C,)), ("ln1_b", (C,)), ("lin1_W", (C, FF)), ("lin1_b", (FF,)),
        ("lin2_W", (FF, C)), ("lin2_b", (C,)), ("ln2_g", (C,)), ("ln2_b", (C,)),
    ]:
        dram_in(nm, shp)
    for k, v in consts.items():
        dram_in(k, v.shape)
    y_own = nc.dram_tensor("y_own", [lq, C], F32, kind="ExternalOutput")
    idx_dram = nc.dram_tensor("idx_scratch", [96 * lq], U16, kind="Internal")
    wp_dram = nc.dram_tensor("wp_scratch", [96 * lq * 4], BF16, kind="Internal")

    with tile.TileContext(nc) as tc:
        with nc.allow_low_precision(reason="float32r/bf16 sampling path"):
            _body(tc, nc, hd, y_own, idx_dram, wp_dram, lq, qc, gelu_hw)
    nc.compile()
    return nc, names, consts


def _body(tc, nc, d, y_own, idx_dram, wp_dram, lq, qc, gelu_hw):
    ACT = nc.scalar
    DVE = nc.vector
    GPS = nc.gpsimd
    NQC = _nchunks(lq, qc)
    NBLK = lq // 128  # 21 own-token blocks

    def ap(nm):
        return d[nm].ap()

    st = tc.alloc_tile_pool(name="wpool", bufs=1)

    def load(nm_or_ap, p, f, tag, dt=F32):
        src = ap(nm_or_ap) if isinstance(nm_or_ap, str) else nm_or_ap
        t = st.tile([p, f], dt, tag=tag, name=tag)
        if dt == MMDT and src.dtype != MMDT:
            src = src.bitcast(MMDT)
        nc.sync.dma_start(t[:, :], src)
        return t

    # ---- persistent weights/consts ----
    wvalf = [load(ap("W_val")[128 * k : 128 * k + 128, :], 128, 256, f"wvalf{k}", dt=MMDT) for k in range(2)]
    wval = [[wvalf[k][:, 128 * m : 128 * m + 128] for m in range(2)] for k in range(2)]
    woffx = [load(AP(d["W_off"], 128 * k * 192, [[192, 128], [2, 96]]), 128, 96, f"woffx{k}", dt=MMDT) for k in range(2)]
    woffy = [load(AP(d["W_off"], 128 * k * 192 + 1, [[192, 128], [2, 96]]), 128, 96, f"woffy{k}", dt=MMDT) for k in range(2)]
    wattn = [load(ap("W_attn")[128 * k : 128 * k + 128, :], 128, 96, f"wattn{k}", dt=MMDT) for k in range(2)]
    woutf = [load(ap("W_out")[128 * k : 128 * k + 128, :], 128, 256, f"woutf{k}", dt=MMDT) for k in range(2)]
    wout = [[woutf[k][:, 128 * m : 128 * m + 128] for m in range(2)] for k in range(2)]
    elx = load("E_LVLX", 3, 96, "elx", dt=MMDT)
    ely = load("E_LVLY", 3, 96, "ely", dt=MMDT)
    esum = load("E_SUM", 96, 8, "esum", dt=MMDT)
    eexp = load("E_EXP", 8, 96, "eexp", dt=MMDT)
    onesc = load("ONESC", 128, 1, "onesc", dt=MMDT)
    eone1 = load("E_ONE1", 1, 128, "eone1", dt=MMDT)
    ident = load("IDENT", 128, 128, "ident")
    cst = load("CONSTS", 96, 8, "cst")
    bvec = lambda nm, m, tag: load(AP(d[nm], 128 * m, [[1, 128], [1, 1]]), 128, 1, tag)
    bval = [bvec("b_val", m, f"bval{m}") for m in range(2)]
    bout = [bvec("b_out", m, f"bout{m}") for m in range(2)]
    l1g = [bvec("ln1_g", m, f"l1g{m}") for m in range(2)]
    l1b = [bvec("ln1_b", m, f"l1b{m}") for m in range(2)]
    l2g = [bvec("ln2_g", m, f"l2g{m}") for m in range(2)]
    l2b = [bvec("ln2_b", m, f"l2b{m}") for m in range(2)]
    lin2b = [bvec("lin2_b", m, f"lin2b{m}") for m in range(2)]
    battn = load(AP(d["b_attn"], 0, [[1, 96], [1, 1]]), 96, 1, "battn")
    boffx_r = load(AP(d["b_off"], 0, [[2, 96], [1, 1]]), 96, 1, "boffxr")
    boffy_r = load(AP(d["b_off"], 1, [[2, 96], [1, 1]]), 96, 1, "boffyr")
    boffx = st.tile([96, 1], F32, tag="boffx", name="boffx")
    boffy = st.tile([96, 1], F32, tag="boffy", name="boffy")
    # cs = pixel_coord + SHIFT - 0.5 (so rint(cs) == floor(pixel)+SHIFT)
    DVE.tensor_scalar_add(boffx[:], boffx_r[:], SHIFT - 1.0)
    DVE.tensor_scalar_add(boffy[:], boffy_r[:], SHIFT - 1.0)
    identB = st.tile([128, 128], BF16, tag="identB", name="identB")
    ACT.activation(identB[:], ident[:], AF.Copy)
    halfc = st.tile([96, 1], F32, tag="halfc", name="halfc")
    nc.any.memset(halfc[:], 0.5)

    idr = lambda: ident[:].bitcast(MMDT)

    # ================= phase A: transposes, value GEMM -> pk, qT =================
    p_pk = tc.alloc_tile_pool(name="ppk", bufs=1)
    pk = [p_pk.tile([128, PKS], F32, tag=f"pk{m}", name=f"pk{m}") for m in range(2)]
    for m in range(2):
        GPS.memset(pk[m][:], 0.0)

    p_own = tc.alloc_tile_pool(name="pown", bufs=1)
    srcOwnT = [p_own.tile([128, lq], F32, tag=f"sot{m}", name=f"sot{m}") for m in range(2)]
    p_q = tc.alloc_tile_pool(name="pq", bufs=1)
    qT = [p_q.tile([128, lq], F32, tag=f"qT{m}", name=f"qT{m}") for m in range(2)]

    p_ld = tc.alloc_tile_pool(name="pld", bufs=1)
    srcf = []
    for i in range(6):
        t = p_ld.tile([128, 1792], F32, tag=f"srcf{i}", name=f"srcf{i}")
        nc.sync.dma_start(t[:], AP(d["src_full"], i * 7 * 128 * 256, [[256, 128], [128 * 256, 7], [1, 256]]))
        srcf.append(t)
    srco = []
    poso = []
    for i in range(3):
        t = p_ld.tile([128, 1792], F32, tag=f"srco{i}", name=f"srco{i}")
        nc.sync.dma_start(t[:], AP(d["src_own"], i * 7 * 128 * 256, [[256, 128], [128 * 256, 7], [1, 256]]))
        srco.append(t)
        t2 = p_ld.tile([128, 1792], F32, tag=f"poso{i}", name=f"poso{i}")
        nc.sync.dma_start(t2[:], AP(d["pos_own"], i * 7 * 128 * 256, [[256, 128], [128 * 256, 7], [1, 256]]))
        poso.append(t2)

    def blk_slice(tiles, b, k):
        # 128-col slice of token-block b, channel half k
        return tiles[b // 7][:, (b % 7) * 256 + 128 * k : (b % 7) * 256 + 128 * k + 128]

    psA = tc.alloc_tile_pool(name="psA", bufs=2, space="PSUM")
    p_tr = tc.alloc_tile_pool(name="ptr", bufs=2)

    # qT = T(src_own + pos_own): transpose-accumulate per block group
    for g0 in range(0, NBLK, 4):
        nb = min(4, NBLK - g0)
        psQ = psA.tile([128, 1024], F32, tag="pst", name="psQ")
        for j in range(nb):
            b = g0 + j
            for k in range(2):
                nc.tensor.matmul(psQ[:, 256 * j + 128 * k : 256 * j + 128 * k + 128].bitcast(MMDT),
                                 blk_slice(srco, b, k).bitcast(MMDT), idr(),
                                 is_transpose=True, start=True, stop=False)
                nc.tensor.matmul(psQ[:, 256 * j + 128 * k : 256 * j + 128 * k + 128].bitcast(MMDT),
                                 blk_slice(poso, b, k).bitcast(MMDT), idr(),
                                 is_transpose=True, start=False, stop=True)
        psQ_v = psQ[:, : 256 * nb].rearrange("p (b2 kk c) -> p b2 kk c", kk=2, c=128)
        for k in range(2):
            ACT.activation(qT[k][:, 128 * g0 : 128 * (g0 + nb)].rearrange("p (b2 c) -> p b2 c", c=128),
                           psQ_v[:, :, k, :], AF.Copy)

    # srcOwnT = T(src_own)
    for g0 in range(0, NBLK, 4):
        nb = min(4, NBLK - g0)
        psS = psA.tile([128, 1024], F32, tag="pst", name="psS")
        for j in range(nb):
            b = g0 + j
            for k in range(2):
                nc.tensor.matmul(psS[:, 256 * j + 128 * k : 256 * j + 128 * k + 128].bitcast(MMDT),
                                 blk_slice(srco, b, k).bitcast(MMDT), idr(),
                                 is_transpose=True, start=True, stop=True)
        psS_v = psS[:, : 256 * nb].rearrange("p (b2 kk c) -> p b2 kk c", kk=2, c=128)
        for k in range(2):
            ACT.activation(srcOwnT[k][:, 128 * g0 : 128 * (g0 + nb)].rearrange("p (b2 c) -> p b2 c", c=128),
                           psS_v[:, :, k, :], AF.Copy)

    # value GEMM (src_full transposed on the fly) -> packed maps
    for (o, n, lvl) in VCHUNKS:
        nb = n // 128
        psT = psA.tile([128, 1024], F32, tag="pst", name="psT")
        for j in range(nb):
            b = (o + 128 * j) // 128
            for k in range(2):
                nc.tensor.matmul(psT[:, 256 * j + 128 * k : 256 * j + 128 * k + 128].bitcast(MMDT),
                                 blk_slice(srcf, b, k).bitcast(MMDT), idr(),
                                 is_transpose=True, start=True, stop=True)
        srcc = [p_tr.tile([128, 512], F32, tag=f"srcc{k}", name=f"srcc{k}") for k in range(2)]
        psT_v = psT[:, : 256 * nb].rearrange("p (b2 kk c) -> p b2 kk c", kk=2, c=128)
        for k in range(2):
            ACT.activation(srcc[k][:, :n].rearrange("p (b2 c) -> p b2 c", c=128),
                           psT_v[:, :, k, :], AF.Copy)
        Wl = SHAPES[lvl][1]
        rs2 = 2 * (Wl + 2)
        y0 = (o - LVLSTART[lvl]) // Wl
        ny = n // Wl
        for m in range(2):
            ps = psA.tile([128, 512], F32, tag="psv", name="psv")
            for k in range(2):
                nc.tensor.matmul(ps[:, :n], wval[k][m].bitcast(MMDT), srcc[k][:, :n].bitcast(MMDT), start=(k == 0), stop=(k == 1))
            pb = pk[m][:].bitcast(BF16)
            base2 = 2 * PK_BASE[lvl]
            v0 = pb[:, base2 + rs2 * (y0 + 1) : base2 + rs2 * (y0 + 1) + ny * rs2].rearrange(
                "p (y c two) -> p y c two", c=Wl + 2, two=2)[:, :, 1 : Wl + 1, 0]
            v1 = pb[:, base2 + rs2 * y0 : base2 + rs2 * y0 + ny * rs2].rearrange(
                "p (y c two) -> p y c two", c=Wl + 2, two=2)[:, :, 1 : Wl + 1, 1]
            ps_v = ps[:, :n].rearrange("p (y x) -> p y x", x=Wl)
            ACT.activation(v0, ps_v, AF.Identity, bias=bval[m][:])
            ACT.activation(v1, ps_v, AF.Identity, bias=bval[m][:])
    p_tr.release()
    psA.release()
    p_ld.release()

    # ========= phase 2: chunked offset/attn math -> idx/weight DRAM =========
    pom = tc.alloc_tile_pool(name="pom", bufs=2)
    pgm = tc.alloc_tile_pool(name="pgm", bufs=2)
    ps2 = tc.alloc_tile_pool(name="ps2", bufs=1, space="PSUM")
    for (o, n) in NQC:
        ps_x = ps2.tile([96, 448], F32, tag="ps_x", name="ps_x")
        ps_y = ps2.tile([96, 448], F32, tag="ps_y", name="ps_y")
        for k in range(2):
            nc.tensor.matmul(ps_x[:, :n], woffx[k][:].bitcast(MMDT), qT[k][:, o : o + n].bitcast(MMDT), start=(k == 0), stop=False)
        refx = pom.tile([3, 448], MMDT, tag="refx", name="refx")
        refy = pom.tile([3, 448], MMDT, tag="refy", name="refy")
        nc.sync.dma_start(refx[:, :n], AP(d["ref_own"], 6 * o, [[2, 3], [6, n]]).bitcast(MMDT))
        nc.sync.dma_start(refy[:, :n], AP(d["ref_own"], 6 * o + 1, [[2, 3], [6, n]]).bitcast(MMDT))
        nc.tensor.matmul(ps_x[:, :n], elx[:].bitcast(MMDT), refx[:, :n].bitcast(MMDT), start=False, stop=True)
        for k in range(2):
            nc.tensor.matmul(ps_y[:, :n], woffy[k][:].bitcast(MMDT), qT[k][:, o : o + n].bitcast(MMDT), start=(k == 0), stop=False)
        nc.tensor.matmul(ps_y[:, :n], ely[:].bitcast(MMDT), refy[:, :n].bitcast(MMDT), start=False, stop=True)

        csx = pom.tile([96, 448], F32, tag="csx", name="csx")
        csy = pom.tile([96, 448], F32, tag="csy", name="csy")
        ACT.activation(csx[:, :n], ps_x[:, :n], AF.Identity, bias=boffx[:])
        ACT.activation(csy[:, :n], ps_y[:, :n], AF.Identity, bias=boffy[:])

        ps_a = ps2.tile([96, 448], F32, tag="ps_x", name="ps_a")
        for k in range(2):
            nc.tensor.matmul(ps_a[:, :n], wattn[k][:].bitcast(MMDT), qT[k][:, o : o + n].bitcast(MMDT), start=(k == 0), stop=(k == 1))
        ex = pom.tile([96, 448], F32, tag="ex", name="ex")
        ACT.activation(ex[:, :n].bitcast(MMDT), ps_a[:, :n], AF.Exp, bias=battn[:])
        ps_d = ps2.tile([8, 448], F32, tag="ps_y", name="ps_d")
        nc.tensor.matmul(ps_d[:, :n], esum[:].bitcast(MMDT), ex[:, :n].bitcast(MMDT), start=True, stop=True)
        rec = pom.tile([8, 448], F32, tag="rec", name="rec")
        DVE.reciprocal(rec[:, :n].bitcast(MMDT), ps_d[:, :n])
        ps_r = ps2.tile([96, 448], F32, tag="ps_x", name="ps_r")
        nc.tensor.matmul(ps_r[:, :n], eexp[:].bitcast(MMDT), rec[:, :n].bitcast(MMDT), start=True, stop=True)
        am = pom.tile([96, 448], F32, tag="am", name="am")
        DVE.tensor_tensor(am[:, :n], ex[:, :n], ps_r[:, :n], OP.mult)

        # validity mask on gpsimd (idle here): am *= (in-range x) * (in-range y)
        mka = pgm.tile([96, 448], F32, tag="mka", name="mka")
        mkb = pgm.tile([96, 448], F32, tag="mkb", name="mkb")
        GPS.tensor_scalar(mka[:, :n], csx[:, :n], cst[:, 3:4], None, OP.is_ge)
        GPS.tensor_scalar(mkb[:, :n], csx[:, :n], cst[:, 4:5], None, OP.is_lt)
        GPS.tensor_tensor(mka[:, :n], mka[:, :n], mkb[:, :n], OP.mult)
        GPS.tensor_scalar(mkb[:, :n], csy[:, :n], cst[:, 3:4], None, OP.is_ge)
        GPS.tensor_tensor(mka[:, :n], mka[:, :n], mkb[:, :n], OP.mult)
        GPS.tensor_scalar(mkb[:, :n], csy[:, :n], cst[:, 5:6], None, OP.is_lt)
        GPS.tensor_tensor(mka[:, :n], mka[:, :n], mkb[:, :n], OP.mult)
        DVE.tensor_tensor(am[:, :n], am[:, :n], mka[:, :n], OP.mult)

        ti = pom.tile([96, 448], I32, tag="ti", name="ti")
        fx = pom.tile([96, 448], F32, tag="fx", name="fx")
        fy = pom.tile([96, 448], F32, tag="fy", name="fy")
        wfx = pom.tile([96, 448], F32, tag="wfx", name="wfx")
        wfy = pom.tile([96, 448], F32, tag="wfy", name="wfy")
        for (coord, fl, wf, hi_clamp) in ((csx, fx, wfx, 1), (csy, fy, wfy, 2)):
            # HW fp32->int32 cast rounds to nearest; rint(cs) == floor(pixel)+SHIFT
            DVE.tensor_copy(ti[:, :n], coord[:, :n])
            DVE.tensor_copy(fl[:, :n], ti[:, :n])
            DVE.tensor_tensor(wf[:, :n], coord[:, :n], fl[:, :n], OP.subtract)  # wx-0.5
            GPS.tensor_scalar(fl[:, :n], fl[:, :n], cst[:, 0:1], None, OP.max)
            GPS.tensor_scalar(fl[:, :n], fl[:, :n], cst[:, hi_clamp : hi_clamp + 1], None, OP.min)
        tmp = pom.tile([96, 448], F32, tag="tmp", name="tmp")
        ACT.activation(tmp[:, :n], fy[:, :n], AF.Identity, bias=cst[:, 7:8], scale=cst[:, 6:7])
        DVE.tensor_tensor(tmp[:, :n], tmp[:, :n], fx[:, :n], OP.add)
        # u16 cast with q_lo-swizzled free order: iu[r, q_lo*(n/16) + q_hi]
        iu = pom.tile([96, 448], U16, tag="iu", name="iu")
        nqh = n // 16
        iu_v = iu[:, :n].rearrange("p (ql qh) -> p ql qh", ql=16)
        tmp_v = tmp[:, :n].rearrange("p (qh ql) -> p ql qh", ql=16)
        DVE.tensor_copy(iu_v, tmp_v)
        # dram layout: row r, position q_lo*QH + q_hi_global
        dst = AP(idx_dram, o // 16, [[lq, 96], [QH, 16], [1, nqh]])
        nc.sync.dma_start(dst, iu_v)

        wxa = pom.tile([96, 448], F32, tag="wxa", name="wxa")
        wxb = pom.tile([96, 448], F32, tag="wxb", name="wxb")
        wya = pom.tile([96, 448], F32, tag="wya", name="wya")
        wyb = pom.tile([96, 448], F32, tag="wyb", name="wyb")
        ACT.activation(wxa[:, :n], wfx[:, :n], AF.Identity, bias=halfc[:], scale=-1.0)  # 1-wx
        ACT.activation(wxb[:, :n], wfx[:, :n], AF.Identity, bias=halfc[:], scale=1.0)   # wx
        ACT.activation(wya[:, :n], wfy[:, :n], AF.Identity, bias=halfc[:], scale=-1.0)
        ACT.activation(wyb[:, :n], wfy[:, :n], AF.Identity, bias=halfc[:], scale=1.0)
        atop = pom.tile([96, 448], F32, tag="atop", name="atop")
        abot = pom.tile([96, 448], F32, tag="abot", name="abot")
        DVE.tensor_tensor(atop[:, :n], am[:, :n], wya[:, :n], OP.mult)
        DVE.tensor_tensor(abot[:, :n], am[:, :n], wyb[:, :n], OP.mult)
        wp = pom.tile([96, 448, 4], BF16, tag="wp", name="wp")
        DVE.tensor_tensor(wp[:, :n, 0], atop[:, :n], wxa[:, :n], OP.mult)
        DVE.tensor_tensor(wp[:, :n, 1], abot[:, :n], wxa[:, :n], OP.mult)
        DVE.tensor_tensor(wp[:, :n, 2], atop[:, :n], wxb[:, :n], OP.mult)
        DVE.tensor_tensor(wp[:, :n, 3], abot[:, :n], wxb[:, :n], OP.mult)
        nc.sync.dma_start(AP(wp_dram, 4 * o, [[4 * lq, 96], [1, 4 * n]]),
                          wp[:, :n, :].rearrange("p a b -> p (a b)"))
    ps2.release()
    pgm.release()
    pom.release()
    p_q.release()

    # ================= phase B: sampling =================
    p_out = tc.alloc_tile_pool(name="pout", bufs=1)
    outT = [p_out.tile([128, lq], F32, tag=f"outT{m}", name=f"outT{m}") for m in range(2)]
    psB = tc.alloc_tile_pool(name="psB", bufs=1, space="PSUM")
    psmp = tc.alloc_tile_pool(name="psamp", bufs=2)
    NCC = lq // qc  # 448-chunks per PSUM bank
    for t in range(2):
        pk_lv = [
            pk[t][:, PK_BASE[l] : PK_BASE[l] + LVLSZ[l]].rearrange("p (a two) -> p a two", two=2)
            for l in range(3)
        ]
        idx16 = psmp.tile([128, 12 * QH], U16, tag="idx16", name="idx16")
        for g in range(8):
            h = 4 * t + g // 2
            src_ap = AP(idx_dram, h * lq, [[QH, 16], [8 * lq, 12], [1, QH]])
            dst_ap = idx16[16 * g : 16 * g + 16, :].rearrange("p (lp qh) -> p lp qh", lp=12)
            nc.sync.dma_start(dst_ap, src_ap)
        accs = [psB.tile([128, qc], F32, tag=f"acc{c}", name=f"acc{c}") for c in range(NCC)]
        for lp in range(12):
            wt = psmp.tile([128, 4 * lq], BF16, tag="wt", name="wt")
            nc.sync.dma_start(wt[:], AP(wp_dram, (8 * lp + 4 * t) * 4 * lq, [[4 * lq, 4], [0, 32], [1, 4 * lq]]))
            g_t = psmp.tile([128, lq, 2], F32, tag="gt", name="gt")
            GPS.indirect_copy(g_t[:], pk_lv[lp // 4], idx16[:, lp * QH : (lp + 1) * QH], i_know_ap_gather_is_preferred=True)
            gb = g_t[:].bitcast(BF16).rearrange("p a b -> p (a b)")
            DVE.tensor_tensor(gb, gb, wt[:], OP.mult)
            pv = g_t[:].bitcast(BF16)  # [128, lq, 4] bf16 taps
            for cix in range(NCC):
                for j in range(4):
                    nc.tensor.matmul(accs[cix][:], identB[:], pv[:, qc * cix : qc * (cix + 1), j],
                                     start=(lp == 0 and j == 0), stop=(lp == 11 and j == 3))
        for cix in range(NCC):
            ACT.activation(outT[t][:, qc * cix : qc * (cix + 1)], accs[cix][:], AF.Copy)
    psmp.release()
    psB.release()

    # ================= phase C: out-proj + LN1, FFN + LN2 -> y =================
    p_fw = tc.alloc_tile_pool(name="pfw", bufs=1)

    def loadf(src, p, f, tag, dt=F32):
        t = p_fw.tile([p, f], dt, tag=tag, name=tag)
        if dt == MMDT and src.dtype != MMDT:
            src = src.bitcast(MMDT)
        nc.sync.dma_start(t[:, :], src)
        return t

    lin1 = [loadf(ap("lin1_W")[128 * k : 128 * k + 128, :], 128, FF, f"lin1{k}", dt=MMDT) for k in range(2)]
    lin2 = loadf(AP(d["lin2_W"], 0, [[256, 128], [128 * 256, 16], [1, 256]]), 128, 16 * 256, "lin2", dt=MMDT)
    lin1b = loadf(AP(d["lin1_b"], 0, [[1, 128], [128, 16]]), 128, 16, "lin1b")

    p_x1 = tc.alloc_tile_pool(name="px1", bufs=1)
    x1 = [p_x1.tile([128, lq], F32, tag=f"x1{m}", name=f"x1{m}") for m in range(2)]
    pres = tc.alloc_tile_pool(name="pres", bufs=1)
    _lnorm(tc, nc, pres, lq, qc, x1, gemm_k=outT, gemm_w=wout, gemm_b=bout,
           resid=srcOwnT, g=l1g, bb=l1b, onesc=onesc, eone1=eone1)
    pres.release()

    x2 = x1
    pffn = tc.alloc_tile_pool(name="pffn", bufs=2)
    ps6 = tc.alloc_tile_pool(name="ps6", bufs=2, space="PSUM")
    ps6b = tc.alloc_tile_pool(name="ps6b", bufs=1, space="PSUM")
    for (o, n) in NQC:
        ps_h2 = [ps6b.tile([128, 448], F32, tag=f"ps_h2{m}", name=f"ps_h2{m}") for m in range(2)]
        for mf in range(16):
            ps_h1 = ps6.tile([128, 448], F32, tag="ps_h1", name="ps_h1")
            for k in range(2):
                nc.tensor.matmul(ps_h1[:, :n], lin1[k][:, 128 * mf : 128 * mf + 128].bitcast(MMDT), x1[k][:, o : o + n].bitcast(MMDT), start=(k == 0), stop=(k == 1))
            h1 = pffn.tile([128, 448], F32, tag="h1", name="h1")
            ACT.activation(h1[:, :n].bitcast(MMDT), ps_h1[:, :n], AF.Gelu if gelu_hw else AF.Tanh, bias=lin1b[:, mf : mf + 1])
            for m in range(2):
                nc.tensor.matmul(ps_h2[m][:, :n], lin2[:, 256 * mf + 128 * m : 256 * mf + 128 * m + 128].bitcast(MMDT), h1[:, :n].bitcast(MMDT), start=(mf == 0), stop=(mf == 15))
        for m in range(2):
            h2s = pffn.tile([128, 448], F32, tag=f"h2s{m}", name=f"h2s{m}")
            ACT.activation(h2s[:, :n], ps_h2[m][:, :n], AF.Identity, bias=lin2b[m][:])
            DVE.tensor_tensor(x2[m][:, o : o + n].bitcast(MMDT), h2s[:, :n], x1[m][:, o : o + n], OP.add)
    # LN2 overlaps the FFN tail (ps6/ps6b still held: 2+2+4 PSUM banks)
    _lnorm(tc, nc, pffn, lq, qc, x2, gemm_k=None, gemm_w=None, gemm_b=None,
           resid=None, g=l2g, bb=l2b, onesc=onesc, eone1=eone1, inplace=x2)
    ps6b.release()
    ps6.release()

    # ---- transpose out (groups of 7 blocks -> one store DMA) ----
    psT2 = tc.alloc_tile_pool(name="psT2", bufs=2, space="PSUM")
    for g0 in range(0, NBLK, 7):
        psY = psT2.tile([128, 1792], F32, tag="psY", name="psY")
        for j in range(7):
            b = g0 + j
            for m in range(2):
                nc.tensor.matmul(psY[:, 256 * j + 128 * m : 256 * j + 128 * m + 128].bitcast(MMDT),
                                 x2[m][:, 128 * b : 128 * b + 128].bitcast(MMDT), idr(),
                                 is_transpose=True, start=True, stop=True)
        yq = pffn.tile([128, 1792], F32, tag="yq", name="yq")
        ACT.activation(yq[:], psY[:], AF.Copy)
        nc.sync.dma_start(AP(y_own, g0 * 128 * 256, [[256, 128], [128 * 256, 7], [1, 256]]), yq[:])
    psT2.release()
    pffn.release()
    p_x1.release()
    p_fw.release()
    p_out.release()
    p_own.release()
    p_pk.release()
    st.release()


def _lnorm(tc, nc, pool, lq, qc, dst, gemm_k, gemm_w, gemm_b, resid, g, bb,
           onesc, eone1, inplace=None, dst_dt=None):
    """dst[m] = LN(x)*g + b over channels (2 x 128 partitions).

    x = resid + (W.T @ gemm_k + gemm_b) when gemm_w given, else x = inplace.
    """
    ACT, DVE = nc.scalar, nc.vector
    psl = tc.alloc_tile_pool(name="psln", bufs=1, space="PSUM")
    eps = pool.tile([1, 1], F32, tag="ln_eps", name="ln_eps")
    nc.any.memset(eps[:], 1e-5)
    for (o, n) in _nchunks(lq, qc):
        xs = []
        if gemm_w is not None:
            for m in range(2):
                ps = psl.tile([128, 448], F32, tag=f"ln_ps{m}", name=f"ln_ps{m}")
                for k in range(2):
                    nc.tensor.matmul(ps[:, :n], gemm_w[k][m].bitcast(MMDT), gemm_k[k][:, o : o + n].bitcast(MMDT), start=(k == 0), stop=(k == 1))
                x = pool.tile([128, 448], F32, tag=f"ln_x{m}", name=f"ln_x{m}")
                ACT.activation(x[:, :n].bitcast(MMDT), ps[:, :n], AF.Identity, bias=gemm_b[m][:])
                DVE.tensor_tensor(x[:, :n].bitcast(MMDT), x[:, :n], resid[m][:, o : o + n], OP.add)
                xs.append(x[:, :n])
        else:
            xs = [inplace[m][:, o : o + n] for m in range(2)]

        ps_s = psl.tile([1, 448], F32, tag="ln_s1", name="ln_s1")
        ps_s2 = psl.tile([1, 448], F32, tag="ln_s2", name="ln_s2")
        sq = pool.tile([128, 448], F32, tag="ln_sq", name="ln_sq")
        for m in range(2):
            nc.tensor.matmul(ps_s[:, :n], onesc[:].bitcast(MMDT), xs[m].bitcast(MMDT), start=(m == 0), stop=(m == 1))
        for m in range(2):
            ACT.activation(sq[:, :n].bitcast(MMDT), xs[m], AF.Square)
            nc.tensor.matmul(ps_s2[:, :n], onesc[:].bitcast(MMDT), sq[:, :n].bitcast(MMDT), start=(m == 0), stop=(m == 1))
        mean = pool.tile([1, 448], F32, tag="ln_mean", name="ln_mean")
        ACT.activation(mean[:, :n].bitcast(MMDT), ps_s[:, :n], AF.Copy, scale=1.0 / 256.0)
        ms = pool.tile([1, 448], F32, tag="ln_ms", name="ln_ms")
        DVE.tensor_tensor(ms[:, :n], mean[:, :n], mean[:, :n], OP.mult)
        var = pool.tile([1, 448], F32, tag="ln_var", name="ln_var")
        ACT.activation(var[:, :n], ps_s2[:, :n], AF.Copy, scale=1.0 / 256.0)
        DVE.tensor_tensor(var[:, :n], var[:, :n], ms[:, :n], OP.subtract)
        std = pool.tile([1, 448], F32, tag="ln_std", name="ln_std")
        ACT.activation(std[:, :n], var[:, :n], AF.Sqrt, bias=eps[:])
        rstd = pool.tile([1, 448], F32, tag="ln_rstd", name="ln_rstd")
        DVE.reciprocal(rstd[:, :n].bitcast(MMDT), std[:, :n])
        ps_bm = psl.tile([128, 448], F32, tag="ln_ps0", name="ln_bm")
        ps_br = psl.tile([128, 448], F32, tag="ln_ps1", name="ln_br")
        nc.tensor.matmul(ps_bm[:, :n], eone1[:].bitcast(MMDT), mean[:, :n].bitcast(MMDT), start=True, stop=True)
        nc.tensor.matmul(ps_br[:, :n], eone1[:].bitcast(MMDT), rstd[:, :n].bitcast(MMDT), start=True, stop=True)
        for m in range(2):
            t = pool.tile([128, 448], F32, tag="ln_t", name="ln_t")
            DVE.tensor_tensor(t[:, :n], xs[m], ps_bm[:, :n], OP.subtract)
            DVE.tensor_tensor(t[:, :n], t[:, :n], ps_br[:, :n], OP.mult)
            ACT.activation(dst[m][:, o : o + n].bitcast(dst_dt or MMDT), t[:, :n], AF.Identity, bias=bb[m][:], scale=g[m][:])
    psl.release()


# ======================= host side =======================

_CACHE = {}


def _get_program():
    if "nc" not in _CACHE:
        nc, names, consts = build_program()
        _CACHE["nc"] = nc
        _CACHE["names"] = names
        _CACHE["consts"] = consts
    return _CACHE["nc"], _CACHE["names"], _CACHE["consts"]


def make_in_maps(inputs, lq=LQ, consts=None):
    consts = consts if consts is not None else _host_consts()
    src = np.ascontiguousarray(np.asarray(inputs["src"], np.float32))
    pos = np.ascontiguousarray(np.asarray(inputs["pos"], np.float32))
    ref = np.ascontiguousarray(np.asarray(inputs["reference_points"], np.float32))
    weights = {
        k: np.ascontiguousarray(np.asarray(inputs[k]), np.float32)
        for k in ["W_val", "b_val", "W_off", "b_off", "W_attn", "b_attn",
                   "W_out", "b_out", "ln1_g", "ln1_b", "lin1_W", "lin1_b",
                   "lin2_W", "lin2_b", "ln2_g", "ln2_b"]
    }
    # permute the 96 (h,l,p) rows into (l,p,h) order
    perm = _perm96()
    perm2 = np.stack([2 * perm, 2 * perm + 1], axis=1).reshape(-1)
    weights["W_off"] = np.ascontiguousarray(weights["W_off"][:, perm2])
    weights["b_off"] = np.ascontiguousarray(weights["b_off"][perm2])
    weights["W_attn"] = np.ascontiguousarray(weights["W_attn"][:, perm])
    weights["b_attn"] = np.ascontiguousarray(weights["b_attn"][perm])
    in_maps = []
    for core in range(8):
        b, half = core // 2, core % 2
        o = half * lq
        m = {
            "src_full": src[b],
            "src_own": np.ascontiguousarray(src[b, o : o + lq]),
            "pos_own": np.ascontiguousarray(pos[b, o : o + lq]),
            "ref_own": np.ascontiguousarray(ref[b, o : o + lq]),
        }
        m.update(weights)
        m.update(consts)
        in_maps.append(m)
    return in_maps


def kernel(**inputs):
    nc, names, consts = _get_program()
    in_maps = make_in_maps(inputs, consts=consts)
    res = run_bass_kernel_spmd(nc, in_maps, core_ids=list(range(8)))
    out = np.zeros((B, L, C), np.float32)
    for core in range(8):
        b, half = core // 2, core % 2
        o = half * LQ
        out[b, o : o + LQ] = res.results[core]["y_own"]
    return out


# revision 23
# speedup vs baseline: 2.5309x; 1.4138x over previous
"""Deformable-attention transformer encoder layer on 8 Trainium2 cores.

Sharding: core = (batch b = core//2, L-half = core%2). Each core computes the
full value map for its batch element (needed for sampling) and runs the rest
of the layer on its 2688-query shard.

All activations are channel-major ([C partitions, seq free]); GEMMs run on
the tensor engine with float32r operands (1 cyc/row at N>=256).
The 96 (head,level,point) rows are laid out as (level,point,head) so the
per-head sampling weights for one (level,point) sit on 4 adjacent rows and
reach the gather's replicated [4x32, q] layout with a single broadcast DMA.
Deformable sampling: value maps are repacked per channel as bf16 vertical
pairs ((v[y,x], v[y+1,x]) in one 4-byte element) with a zero border, so one
GPSIMD indirect_copy index fetches all 4 bilinear taps of a point; gathers run
full-length (one per (half, level-point)). The 4 bf16 tap weights fold
attention/bilinear/validity; DVE applies them in one in-place bf16 multiply
and the tap/point accumulation runs on the tensor engine as bf16
identity-matmuls into PSUM.

Note: the fp32->int cast (floor) compensates for HW round-to-nearest; CoreSim
truncates instead, so CoreSim outputs diverge from hardware (hardware is the
reference vs the fp32 oracle).
"""

import sys

for _p in ("/opt/trn_rl_repo",):
    if _p not in sys.path:
        sys.path.insert(0, _p)

import numpy as np
import concourse.bass as bass
import concourse.mybir as mybir
import concourse.tile as tile
from concourse import bacc
from concourse.bass import AP
from concourse.bass_utils import run_bass_kernel_spmd

F32 = mybir.dt.float32
MMDT = mybir.dt.float32r  # matmul operand dtype (bit-identical to f32)
BF16 = mybir.dt.bfloat16
I32 = mybir.dt.int32
I16 = mybir.dt.int16
U16 = mybir.dt.uint16
AF = mybir.ActivationFunctionType
OP = mybir.AluOpType
AX = mybir.AxisListType

B, L, C = 4, 5376, 256
NH, NL, NP = 8, 3, 4
FF = 2048
SHAPES = [(64, 64), (32, 32), (16, 16)]
LVLSTART = [0, 4096, 5120]
SHIFT = 16.0  # added to pixel coords so floor == int-trunc
LQ = L // 2
QH = LQ // 16  # 168

# packed-map geometry: per level rows H+1 (y0 in [-1,H-1]), cols W+2 (x in [-1,W])
PK_BASE = []
_acc = 0
for _h, _w in SHAPES:
    PK_BASE.append(_acc)
    _acc += (_h + 1) * (_w + 2)
PKS = _acc + (_acc % 2) + 2  # even + safety pad
LVLSZ = [(h + 1) * (w + 2) for h, w in SHAPES]

# value-GEMM chunks, level aligned (tokens o..o+n inside one level)
VCHUNKS = (
    [(o, 512, 0) for o in range(0, 4096, 512)]
    + [(o, 512, 1) for o in range(4096, 5120, 512)]
    + [(5120, 256, 2)]
)


def _row_lph(r):
    # partition r -> (level, point, head) with r = (l*4+p)*8 + h
    return r // 32, (r // 8) % 4, r % 8


def _host_consts():
    W = np.zeros(96, np.float32)
    H = np.zeros(96, np.float32)
    WP2 = np.zeros(96, np.float32)
    KIDX = np.zeros(96, np.float32)
    for r in range(96):
        lvl, _, _ = _row_lph(r)
        h, w = SHAPES[lvl]
        W[r], H[r] = w, h
        WP2[r] = w + 2
        # level-relative: the gather uses a per-level slice of the packed map
        KIDX[r] = -(SHIFT - 1.0) * (w + 2) - (SHIFT - 1.0)
    c = {}
    c["CONSTS"] = np.stack(
        [
            np.full(96, SHIFT - 1.0, np.float32),  # 0: clamp lo for floor
            W + SHIFT - 1.0,                       # 1: clamp hi x
            H + SHIFT - 1.0,                       # 2: clamp hi y
            np.full(96, SHIFT - 1.5, np.float32),  # 3: mask lo (on cs)
            W + SHIFT - 0.5,                       # 4: mask hi x (on cs)
            H + SHIFT - 0.5,                       # 5: mask hi y (on cs)
            WP2,                                   # 6
            KIDX,                                  # 7
            KIDX + 1.0,                            # 8
            np.zeros(96, np.float32),              # 9 (pad)
        ],
        axis=1,
    ).astype(np.float32)
    ELX = np.zeros((3, 96), np.float32)
    ELY = np.zeros((3, 96), np.float32)
    for r in range(96):
        lvl, _, _ = _row_lph(r)
        ELX[lvl, r] = SHAPES[lvl][1]
        ELY[lvl, r] = SHAPES[lvl][0]
    c["E_LVLX"], c["E_LVLY"] = ELX, ELY
    ES = np.zeros((96, 8), np.float32)
    EE = np.zeros((8, 96), np.float32)
    for r in range(96):
        ES[r, r % 8] = 1.0
        EE[r % 8, r] = 1.0
    c["E_SUM"], c["E_EXP"] = ES, EE
    c["ONESC"] = np.ones((128, 1), np.float32)
    c["E_ONE1"] = np.ones((1, 128), np.float32)
    c["IDENT"] = np.eye(128, dtype=np.float32)
    return c


def _perm96():
    # perm[r] = original (h,l,p) index for new row r = (l*4+p)*8 + h
    perm = np.zeros(96, np.int64)
    for r in range(96):
        lvl, p, h = _row_lph(r)
        perm[r] = h * 12 + lvl * 4 + p
    return perm


def _nchunks(n, step):
    out, i = [], 0
    while i < n:
        out.append((i, min(step, n - i)))
        i += step
    return out


def build_program(lq=LQ, qc=448, gelu_hw=True):
    nc = bacc.Bacc("TRN2", target_bir_lowering=False, debug=False)
    consts = _host_consts()

    hd = {}
    names = []
    def dram_in(name, shape):
        hd[name] = nc.dram_tensor(name, list(shape), F32, kind="ExternalInput")
        names.append(name)
        return hd[name]

    for nm, shp in [
        ("src_full", (L, C)), ("src_own", (lq, C)), ("pos_own", (lq, C)),
        ("ref_own", (lq, NL, 2)),
        ("W_val", (C, C)), ("b_val", (C,)), ("W_off", (C, 192)), ("b_off", (192,)),
        ("W_attn", (C, 96)), ("b_attn", (96,)), ("W_out", (C, C)), ("b_out", (C,)),
        ("ln1_g", (C,)), ("ln1_b", (C,)), ("lin1_W", (C, FF)), ("lin1_b", (FF,)),
        ("lin2_W", (FF, C)), ("lin2_b", (C,)), ("ln2_g", (C,)), ("ln2_b", (C,)),
    ]:
        dram_in(nm, shp)
    for k, v in consts.items():
        dram_in(k, v.shape)
    y_own = nc.dram_tensor("y_own", [lq, C], F32, kind="ExternalOutput")
    idx_dram = nc.dram_tensor("idx_scratch", [96 * 2 * lq], I16, kind="Internal")
    wp_dram = nc.dram_tensor("wp_scratch", [96 * lq * 4], BF16, kind="Internal")

    with tile.TileContext(nc) as tc:
        with nc.allow_low_precision(reason="float32r/bf16 sampling path"):
            _body(tc, nc, hd, y_own, idx_dram, wp_dram, lq, qc, gelu_hw)
    nc.compile()
    return nc, names, consts


def _body(tc, nc, d, y_own, idx_dram, wp_dram, lq, qc, gelu_hw):
    ACT = nc.scalar
    DVE = nc.vector
    GPS = nc.gpsimd
    NQC = _nchunks(lq, qc)
    NBLK = lq // 128  # 21 own-token blocks

    def ap(nm):
        return d[nm].ap()

    st = tc.alloc_tile_pool(name="wpool", bufs=1)

    def load(nm_or_ap, p, f, tag, dt=F32):
        src = ap(nm_or_ap) if isinstance(nm_or_ap, str) else nm_or_ap
        t = st.tile([p, f], dt, tag=tag, name=tag)
        if dt == MMDT and src.dtype != MMDT:
            src = src.bitcast(MMDT)
        nc.sync.dma_start(t[:, :], src)
        return t

    # data pools + first loads (emitted before the weight DMAs: the SP queue
    # is in-order and the transposes need src/pos first)
    p_pk = tc.alloc_tile_pool(name="ppk", bufs=1)
    pk = [p_pk.tile([128, PKS], F32, tag=f"pk{m}", name=f"pk{m}") for m in range(2)]
    for m in range(2):
        GPS.memset(pk[m][:], 0.0)
    p_own = tc.alloc_tile_pool(name="pown", bufs=1)
    srcOwnT = [p_own.tile([128, lq], F32, tag=f"sot{m}", name=f"sot{m}") for m in range(2)]
    p_q = tc.alloc_tile_pool(name="pq", bufs=1)
    qT = [p_q.tile([128, lq], F32, tag=f"qT{m}", name=f"qT{m}") for m in range(2)]
    p_ld = tc.alloc_tile_pool(name="pld", bufs=1)
    srcf_t = {}
    srco_t = {}
    poso_t = {}

    def _ld_tile(cache, tag, bufs, dname, i):
        if i not in cache:
            t = p_ld.tile([128, 1792], MMDT, tag=tag, name=f"{tag}{i}", bufs=bufs)
            nc.sync.dma_start(t[:], AP(d[dname], i * 7 * 128 * 256, [[256, 128], [128 * 256, 7], [1, 256]]).bitcast(MMDT))
            cache[i] = t
        return cache[i]

    def blk_slice(cache, tag, bufs, dname, b, k):
        t = _ld_tile(cache, tag, bufs, dname, b // 7)
        return t[:, (b % 7) * 256 + 128 * k : (b % 7) * 256 + 128 * k + 128]

    _ld_tile(srco_t, "srco", 2, "src_own", 0)
    _ld_tile(poso_t, "poso", 2, "pos_own", 0)
    _ld_tile(srcf_t, "srcf", 3, "src_full", 0)

    # ---- persistent weights/consts ----
    wvalf = [load(ap("W_val")[128 * k : 128 * k + 128, :], 128, 256, f"wvalf{k}", dt=MMDT) for k in range(2)]
    wval = [[wvalf[k][:, 128 * m : 128 * m + 128] for m in range(2)] for k in range(2)]
    woffx = [load(AP(d["W_off"], 128 * k * 192, [[192, 128], [2, 96]]), 128, 96, f"woffx{k}", dt=MMDT) for k in range(2)]
    woffy = [load(AP(d["W_off"], 128 * k * 192 + 1, [[192, 128], [2, 96]]), 128, 96, f"woffy{k}", dt=MMDT) for k in range(2)]
    wattn = [load(ap("W_attn")[128 * k : 128 * k + 128, :], 128, 96, f"wattn{k}", dt=MMDT) for k in range(2)]
    woutf = [load(ap("W_out")[128 * k : 128 * k + 128, :], 128, 256, f"woutf{k}", dt=MMDT) for k in range(2)]
    wout = [[woutf[k][:, 128 * m : 128 * m + 128] for m in range(2)] for k in range(2)]
    elx = load("E_LVLX", 3, 96, "elx", dt=MMDT)
    ely = load("E_LVLY", 3, 96, "ely", dt=MMDT)
    esum = load("E_SUM", 96, 8, "esum", dt=MMDT)
    eexp = load("E_EXP", 8, 96, "eexp", dt=MMDT)
    onesc = load("ONESC", 128, 1, "onesc", dt=MMDT)
    eone1 = load("E_ONE1", 1, 128, "eone1", dt=MMDT)
    ident = load("IDENT", 128, 128, "ident", dt=MMDT)
    cst = load("CONSTS", 96, 10, "cst")
    bvec = lambda nm, m, tag: load(AP(d[nm], 128 * m, [[1, 128], [1, 1]]), 128, 1, tag)
    bval = [bvec("b_val", m, f"bval{m}") for m in range(2)]
    bout = [bvec("b_out", m, f"bout{m}") for m in range(2)]
    l1g = [bvec("ln1_g", m, f"l1g{m}") for m in range(2)]
    l1b = [bvec("ln1_b", m, f"l1b{m}") for m in range(2)]
    l2g = [bvec("ln2_g", m, f"l2g{m}") for m in range(2)]
    l2b = [bvec("ln2_b", m, f"l2b{m}") for m in range(2)]
    lin2b = [bvec("lin2_b", m, f"lin2b{m}") for m in range(2)]
    battn = load(AP(d["b_attn"], 0, [[1, 96], [1, 1]]), 96, 1, "battn")
    boffx_r = load(AP(d["b_off"], 0, [[2, 96], [1, 1]]), 96, 1, "boffxr")
    boffy_r = load(AP(d["b_off"], 1, [[2, 96], [1, 1]]), 96, 1, "boffyr")
    boffx = st.tile([96, 1], F32, tag="boffx", name="boffx")
    boffy = st.tile([96, 1], F32, tag="boffy", name="boffy")
    # cs = pixel_coord + SHIFT - 0.5 (so rint(cs) == floor(pixel)+SHIFT)
    DVE.tensor_scalar_add(boffx[:], boffx_r[:], SHIFT - 1.0)
    DVE.tensor_scalar_add(boffy[:], boffy_r[:], SHIFT - 1.0)
    identB = st.tile([128, 128], BF16, tag="identB", name="identB")
    ACT.activation(identB[:], ident[:].bitcast(F32), AF.Copy)
    halfc = st.tile([96, 1], F32, tag="halfc", name="halfc")
    nc.any.memset(halfc[:], 0.5)

    idr = lambda: ident[:].bitcast(MMDT)

    # ================= phase A: transposes, value GEMM -> pk, qT =================
    # PSUM: psA (pst 2 banks + psv 2) and ps2 (4 banks) coexist so phase 2
    # overlaps the value GEMM.
    psA = tc.alloc_tile_pool(name="psA", bufs=1, space="PSUM")
    ps2 = tc.alloc_tile_pool(name="ps2", bufs=1, space="PSUM")
    p_tr = tc.alloc_tile_pool(name="ptr", bufs=2)

    # qT = T(src_own + pos_own) and srcOwnT = T(src_own), one pass per group
    def qt_group(g0):
        nb = min(4, NBLK - g0)
        psQ = psA.tile([128, 1024], F32, tag="pst", name="psQ")
        psS = psA.tile([128, 1024], F32, tag="pst2", name="psS")
        for j in range(nb):
            b = g0 + j
            for k in range(2):
                so = blk_slice(srco_t, "srco", 2, "src_own", b, k)
                po = blk_slice(poso_t, "poso", 2, "pos_own", b, k)
                nc.tensor.matmul(psS[:, 256 * j + 128 * k : 256 * j + 128 * k + 128].bitcast(MMDT),
                                 so.bitcast(MMDT), idr(),
                                 is_transpose=True, start=True, stop=True)
                nc.tensor.matmul(psQ[:, 256 * j + 128 * k : 256 * j + 128 * k + 128].bitcast(MMDT),
                                 so.bitcast(MMDT), idr(),
                                 is_transpose=True, start=True, stop=False)
                nc.tensor.matmul(psQ[:, 256 * j + 128 * k : 256 * j + 128 * k + 128].bitcast(MMDT),
                                 po.bitcast(MMDT), idr(),
                                 is_transpose=True, start=False, stop=True)
        psQ_v = psQ[:, : 256 * nb].rearrange("p (b2 kk c) -> p b2 kk c", kk=2, c=128)
        psS_v = psS[:, : 256 * nb].rearrange("p (b2 kk c) -> p b2 kk c", kk=2, c=128)
        for k in range(2):
            ACT.activation(qT[k][:, 128 * g0 : 128 * (g0 + nb)].bitcast(MMDT).rearrange("p (b2 c) -> p b2 c", c=128),
                           psQ_v[:, :, k, :], AF.Copy)
            ACT.activation(srcOwnT[k][:, 128 * g0 : 128 * (g0 + nb)].rearrange("p (b2 c) -> p b2 c", c=128),
                           psS_v[:, :, k, :], AF.Copy)

    # value GEMM (src_full transposed on the fly) -> packed maps
    def v_chunk(o, n, lvl):
        nb = n // 128
        psT = psA.tile([128, 1024], F32, tag="pst", name="psT")
        for j in range(nb):
            b = (o + 128 * j) // 128
            for k in range(2):
                nc.tensor.matmul(psT[:, 256 * j + 128 * k : 256 * j + 128 * k + 128].bitcast(MMDT),
                                 blk_slice(srcf_t, "srcf", 3, "src_full", b, k).bitcast(MMDT), idr(),
                                 is_transpose=True, start=True, stop=True)
        srcc = [p_tr.tile([128, 512], F32, tag=f"srcc{k}", name=f"srcc{k}") for k in range(2)]
        psT_v = psT[:, : 256 * nb].rearrange("p (b2 kk c) -> p b2 kk c", kk=2, c=128)
        for k in range(2):
            ACT.activation(srcc[k][:, :n].bitcast(MMDT).rearrange("p (b2 c) -> p b2 c", c=128),
                           psT_v[:, :, k, :], AF.Copy)
        Wl = SHAPES[lvl][1]
        rs2 = 2 * (Wl + 2)
        y0 = (o - LVLSTART[lvl]) // Wl
        ny = n // Wl
        for m in range(2):
            ps = psA.tile([128, 512], F32, tag="psv", name="psv", bufs=1)
            for k in range(2):
                nc.tensor.matmul(ps[:, :n], wval[k][m].bitcast(MMDT), srcc[k][:, :n].bitcast(MMDT), start=(k == 0), stop=(k == 1))
            pb = pk[m][:].bitcast(BF16)
            base2 = 2 * PK_BASE[lvl]
            v0 = pb[:, base2 + rs2 * (y0 + 1) : base2 + rs2 * (y0 + 1) + ny * rs2].rearrange(
                "p (y c two) -> p y c two", c=Wl + 2, two=2)[:, :, 1 : Wl + 1, 0]
            v1 = pb[:, base2 + rs2 * y0 : base2 + rs2 * y0 + ny * rs2].rearrange(
                "p (y c two) -> p y c two", c=Wl + 2, two=2)[:, :, 1 : Wl + 1, 1]
            ps_v = ps[:, :n].rearrange("p (y x) -> p y x", x=Wl)
            ACT.activation(v0, ps_v, AF.Identity, bias=bval[m][:])
            ACT.activation(v1, ps_v, AF.Identity, bias=bval[m][:])

    # ========= phase 2: chunked offset/attn math -> idx/weight DRAM =========
    # (emitted interleaved with the value GEMM so both run concurrently:
    # separate PSUM banks, DVE/Pool vs the GEMM's PE/Act/DMA)
    pom = tc.alloc_tile_pool(name="pom", bufs=1)
    pgm = tc.alloc_tile_pool(name="pgm", bufs=1)

    def p2_chunk(o, n):
        ps_x = ps2.tile([96, 448], F32, tag="ps_x", name="ps_x")
        ps_y = ps2.tile([96, 448], F32, tag="ps_y", name="ps_y")
        for k in range(2):
            nc.tensor.matmul(ps_x[:, :n], woffx[k][:].bitcast(MMDT), qT[k][:, o : o + n].bitcast(MMDT), start=(k == 0), stop=False)
        refx = pom.tile([3, 448], MMDT, tag="refx", name="refx")
        refy = pom.tile([3, 448], MMDT, tag="refy", name="refy")
        nc.sync.dma_start(refx[:, :n], AP(d["ref_own"], 6 * o, [[2, 3], [6, n]]).bitcast(MMDT))
        nc.sync.dma_start(refy[:, :n], AP(d["ref_own"], 6 * o + 1, [[2, 3], [6, n]]).bitcast(MMDT))
        nc.tensor.matmul(ps_x[:, :n], elx[:].bitcast(MMDT), refx[:, :n].bitcast(MMDT), start=False, stop=True)
        for k in range(2):
            nc.tensor.matmul(ps_y[:, :n], woffy[k][:].bitcast(MMDT), qT[k][:, o : o + n].bitcast(MMDT), start=(k == 0), stop=False)
        nc.tensor.matmul(ps_y[:, :n], ely[:].bitcast(MMDT), refy[:, :n].bitcast(MMDT), start=False, stop=True)

        csx = pom.tile([96, 448], F32, tag="csx", name="csx", bufs=2)
        csy = pom.tile([96, 448], F32, tag="csy", name="csy", bufs=2)
        ACT.activation(csx[:, :n], ps_x[:, :n], AF.Identity, bias=boffx[:])
        ACT.activation(csy[:, :n], ps_y[:, :n], AF.Identity, bias=boffy[:])

        ps_a = ps2.tile([96, 448], F32, tag="psar", name="ps_a")
        for k in range(2):
            nc.tensor.matmul(ps_a[:, :n], wattn[k][:].bitcast(MMDT), qT[k][:, o : o + n].bitcast(MMDT), start=(k == 0), stop=(k == 1))
        ex = pom.tile([96, 448], F32, tag="ex", name="ex", bufs=2)
        ACT.activation(ex[:, :n].bitcast(MMDT), ps_a[:, :n], AF.Exp, bias=battn[:])
        ps_d = ps2.tile([8, 448], F32, tag="ps_y", name="ps_d")
        nc.tensor.matmul(ps_d[:, :n], esum[:].bitcast(MMDT), ex[:, :n].bitcast(MMDT), start=True, stop=True)
        rec = pom.tile([8, 448], F32, tag="rec", name="rec")
        DVE.reciprocal(rec[:, :n].bitcast(MMDT), ps_d[:, :n])
        ps_r = ps2.tile([96, 448], F32, tag="psar", name="ps_r")
        nc.tensor.matmul(ps_r[:, :n], eexp[:].bitcast(MMDT), rec[:, :n].bitcast(MMDT), start=True, stop=True)
        am = pom.tile([96, 448], F32, tag="am", name="am", bufs=2)
        DVE.tensor_tensor(am[:, :n], ex[:, :n], ps_r[:, :n], OP.mult)

        # validity mask on gpsimd (idle here): am *= (in-range x) * (in-range y)
        mka = pgm.tile([96, 448], F32, tag="mka", name="mka")
        mkb = pgm.tile([96, 448], F32, tag="mkb", name="mkb")
        GPS.tensor_scalar(mka[:, :n], csx[:, :n], cst[:, 3:4], None, OP.is_ge)
        GPS.tensor_scalar(mkb[:, :n], csx[:, :n], cst[:, 4:5], None, OP.is_lt)
        GPS.tensor_tensor(mka[:, :n], mka[:, :n], mkb[:, :n], OP.mult)
        GPS.tensor_scalar(mkb[:, :n], csy[:, :n], cst[:, 3:4], None, OP.is_ge)
        GPS.tensor_tensor(mka[:, :n], mka[:, :n], mkb[:, :n], OP.mult)
        GPS.tensor_scalar(mkb[:, :n], csy[:, :n], cst[:, 5:6], None, OP.is_lt)
        GPS.tensor_tensor(mka[:, :n], mka[:, :n], mkb[:, :n], OP.mult)
        DVE.tensor_tensor(am[:, :n], am[:, :n], mka[:, :n], OP.mult)

        ti = pom.tile([96, 448], I32, tag="ti", name="ti")
        fx = pom.tile([96, 448], F32, tag="fx", name="fx", bufs=2)
        fy = pom.tile([96, 448], F32, tag="fy", name="fy", bufs=2)
        wfx = pom.tile([96, 448], F32, tag="wfx", name="wfx", bufs=2)
        wfy = pom.tile([96, 448], F32, tag="wfy", name="wfy", bufs=2)
        for (coord, tt, fl, wf, hi_clamp) in ((csx, ti, fx, wfx, 1), (csy, ti, fy, wfy, 2)):
            # HW fp32->int32 cast rounds to nearest; rint(cs) == floor(pixel)+SHIFT
            DVE.tensor_copy(tt[:, :n], coord[:, :n])
            # fused i32->f32 convert + clamp (clamp changes the floor only for
            # masked points, so the weight wf below may use the clamped value)
            GPS.tensor_scalar(fl[:, :n], tt[:, :n], cst[:, 0:1], cst[:, hi_clamp : hi_clamp + 1], OP.max, OP.min)
            DVE.tensor_tensor(wf[:, :n], coord[:, :n], fl[:, :n], OP.subtract)  # wx-0.5
        tmp = pom.tile([96, 448], F32, tag="tmp", name="tmp", bufs=2)
        DVE.scalar_tensor_tensor(tmp[:, :n], fy[:, :n], cst[:, 6:7], fx[:, :n], OP.mult, OP.add)
        # interleaved (left, right) indices, i16, q_lo-swizzled for the 16-wrap:
        # flat position Q = 2q+r; iu2[p, ql, qh] = index for Q = qh*16+ql
        iu2 = pom.tile([96, 2 * 448], I16, tag="iu2", name="iu2")
        nqh2 = 2 * n // 16
        iu2_v = iu2[:, : 2 * n].rearrange("p (ql qh) -> p ql qh", ql=16)
        tmp_v = tmp[:, :n].rearrange("p (qh m) -> p m qh", m=8)
        DVE.tensor_scalar(iu2_v[:, 0::2, :], tmp_v, cst[:, 7:8], None, OP.add)
        DVE.tensor_scalar(iu2_v[:, 1::2, :], tmp_v, cst[:, 8:9], None, OP.add)
        dst = AP(idx_dram, 2 * o // 16, [[2 * lq, 96], [2 * lq // 16, 16], [1, nqh2]])
        nc.sync.dma_start(dst, iu2_v)

        wxa = pom.tile([96, 448], F32, tag="wxa", name="wxa")
        wxb = pom.tile([96, 448], F32, tag="wxb", name="wxb")
        wya = pom.tile([96, 448], F32, tag="wya", name="wya")
        wyb = pom.tile([96, 448], F32, tag="wyb", name="wyb")
        ACT.activation(wxa[:, :n], wfx[:, :n], AF.Identity, bias=halfc[:], scale=-1.0)  # 1-wx
        ACT.activation(wxb[:, :n], wfx[:, :n], AF.Identity, bias=halfc[:], scale=1.0)   # wx
        ACT.activation(wya[:, :n], wfy[:, :n], AF.Identity, bias=halfc[:], scale=-1.0)
        ACT.activation(wyb[:, :n], wfy[:, :n], AF.Identity, bias=halfc[:], scale=1.0)
        atop, abot = wya, wyb
        DVE.tensor_tensor(atop[:, :n], am[:, :n], wya[:, :n], OP.mult)
        DVE.tensor_tensor(abot[:, :n], am[:, :n], wyb[:, :n], OP.mult)
        wp = pom.tile([96, 448, 4], BF16, tag="wp", name="wp")
        DVE.tensor_tensor(wp[:, :n, 0], atop[:, :n], wxa[:, :n], OP.mult)
        DVE.tensor_tensor(wp[:, :n, 1], abot[:, :n], wxa[:, :n], OP.mult)
        DVE.tensor_tensor(wp[:, :n, 2], atop[:, :n], wxb[:, :n], OP.mult)
        DVE.tensor_tensor(wp[:, :n, 3], abot[:, :n], wxb[:, :n], OP.mult)
        nc.sync.dma_start(AP(wp_dram, 4 * o, [[4 * lq, 96], [1, 4 * n]]),
                          wp[:, :n, :].rearrange("p a b -> p (a b)"))

    # qT group g completes queries < 512(g+1); p2 chunk c needs < 448(c+1)
    qt_group(0)
    qt_group(4)
    vi = 0
    for ci in range(len(NQC)):
        g0 = 8 + 4 * ci
        if g0 < NBLK:
            qt_group(g0)
        p2_chunk(*NQC[ci])
        # a couple of value-GEMM chunks between phase-2 chunks keeps PE fed
        for _ in range(2):
            if vi < len(VCHUNKS):
                v_chunk(*VCHUNKS[vi])
                vi += 1
    while vi < len(VCHUNKS):
        v_chunk(*VCHUNKS[vi])
        vi += 1
    pgm.release()
    pom.release()
    p_tr.release()
    ps2.release()
    psA.release()
    p_ld.release()
    p_q.release()

    # ================= phase B: sampling =================
    p_out = tc.alloc_tile_pool(name="pout", bufs=1)
    outT = [p_out.tile([128, lq], F32, tag=f"outT{m}", name=f"outT{m}") for m in range(2)]
    psB = tc.alloc_tile_pool(name="psB", bufs=1, space="PSUM")
    psmp = tc.alloc_tile_pool(name="psamp", bufs=2)
    NCC = lq // qc  # 448-chunks per PSUM bank
    for t in range(2):
        pk_lv = [pk[t][:, PK_BASE[l] : PK_BASE[l] + LVLSZ[l]] for l in range(3)]
        QH2 = 2 * lq // 16
        idx16 = psmp.tile([128, 12 * QH2], I16, tag="idx16", name="idx16", bufs=1)
        for g in range(8):
            h = 4 * t + g // 2
            src_ap = AP(idx_dram, h * 2 * lq, [[QH2, 16], [8 * 2 * lq, 12], [1, QH2]])
            dst_ap = idx16[16 * g : 16 * g + 16, :].rearrange("p (lp qh) -> p lp qh", lp=12)
            nc.sync.dma_start(dst_ap, src_ap)
        accs = [psB.tile([128, qc], F32, tag=f"acc{c}", name=f"acc{c}") for c in range(NCC)]
        for lp in range(12):
            wth = []
            for hh in range(2):
                wt = psmp.tile([128, 2 * lq], BF16, tag="wt", name="wt", bufs=3)
                nc.sync.dma_start(wt[:], AP(wp_dram, (8 * lp + 4 * t) * 4 * lq + hh * 2 * lq,
                                            [[4 * lq, 4], [0, 32], [1, 2 * lq]]))
                wth.append(wt)
            g_t = psmp.tile([128, lq, 2], F32, tag="gt", name="gt", bufs=3)
            GPS.ap_gather(g_t[:].rearrange("p a b -> p (a b)"), pk_lv[lp // 4],
                          idx16[:, lp * QH2 : (lp + 1) * QH2],
                          channels=128, num_elems=LVLSZ[lp // 4], d=1, num_idxs=2 * lq)
            gb = g_t[:].bitcast(BF16).rearrange("p a b -> p (a b)")
            for hh in range(2):
                DVE.tensor_tensor(gb[:, hh * 2 * lq : (hh + 1) * 2 * lq], gb[:, hh * 2 * lq : (hh + 1) * 2 * lq], wth[hh][:], OP.mult)
            pv = g_t[:].bitcast(BF16)  # [128, lq, 4] bf16 taps
            for cix in range(NCC):
                for j in range(4):
                    nc.tensor.matmul(accs[cix][:], identB[:], pv[:, qc * cix : qc * (cix + 1), j],
                                     start=(lp == 0 and j == 0), stop=(lp == 11 and j == 3))
        for cix in range(NCC):
            ACT.activation(outT[t][:, qc * cix : qc * (cix + 1)].bitcast(MMDT), accs[cix][:], AF.Copy)
    psmp.release()
    psB.release()

    # ================= phase C: out-proj + LN1, FFN + LN2 -> y =================
    p_fw = tc.alloc_tile_pool(name="pfw", bufs=1)

    def loadf(src, p, f, tag, dt=F32):
        t = p_fw.tile([p, f], dt, tag=tag, name=tag)
        if dt == MMDT and src.dtype != MMDT:
            src = src.bitcast(MMDT)
        nc.sync.dma_start(t[:, :], src)
        return t

    lin1 = [loadf(ap("lin1_W")[128 * k : 128 * k + 128, :], 128, FF, f"lin1{k}", dt=MMDT) for k in range(2)]
    lin2 = loadf(AP(d["lin2_W"], 0, [[256, 128], [128 * 256, 16], [1, 256]]), 128, 16 * 256, "lin2", dt=MMDT)
    lin1b = loadf(AP(d["lin1_b"], 0, [[1, 128], [128, 16]]), 128, 16, "lin1b")

    p_x1 = tc.alloc_tile_pool(name="px1", bufs=1)
    x1 = [p_x1.tile([128, lq], F32, tag=f"x1{m}", name=f"x1{m}") for m in range(2)]
    pres = tc.alloc_tile_pool(name="pres", bufs=2)
    _lnorm(tc, nc, pres, lq, qc, x1, gemm_k=outT, gemm_w=wout, gemm_b=bout,
           resid=srcOwnT, g=l1g, bb=l1b, onesc=onesc, eone1=eone1)
    pres.release()

    x2 = x1
    pffn = tc.alloc_tile_pool(name="pffn", bufs=2)
    ps6 = tc.alloc_tile_pool(name="ps6", bufs=2, space="PSUM")
    ps6b = tc.alloc_tile_pool(name="ps6b", bufs=1, space="PSUM")
    for (o, n) in NQC:
        ps_h2 = [ps6b.tile([128, 448], F32, tag=f"ps_h2{m}", name=f"ps_h2{m}") for m in range(2)]
        for mf in range(16):
            ps_h1 = ps6.tile([128, 448], F32, tag="ps_h1", name="ps_h1")
            for k in range(2):
                nc.tensor.matmul(ps_h1[:, :n], lin1[k][:, 128 * mf : 128 * mf + 128].bitcast(MMDT), x1[k][:, o : o + n].bitcast(MMDT), start=(k == 0), stop=(k == 1))
            h1 = pffn.tile([128, 448], F32, tag="h1", name="h1")
            ACT.activation(h1[:, :n].bitcast(MMDT), ps_h1[:, :n], AF.Gelu if gelu_hw else AF.Tanh, bias=lin1b[:, mf : mf + 1])
            for m in range(2):
                nc.tensor.matmul(ps_h2[m][:, :n], lin2[:, 256 * mf + 128 * m : 256 * mf + 128 * m + 128].bitcast(MMDT), h1[:, :n].bitcast(MMDT), start=(mf == 0), stop=(mf == 15))
        for m in range(2):
            DVE.scalar_tensor_tensor(x2[m][:, o : o + n].bitcast(MMDT), ps_h2[m][:, :n],
                                     lin2b[m][:], x1[m][:, o : o + n], OP.add, OP.add)
    # LN2 overlaps the FFN tail (ps6/ps6b still held: 2+2+4 PSUM banks)
    _lnorm(tc, nc, pffn, lq, qc, x2, gemm_k=None, gemm_w=None, gemm_b=None,
           resid=None, g=l2g, bb=l2b, onesc=onesc, eone1=eone1, inplace=x2)
    ps6b.release()
    ps6.release()

    # ---- transpose out (groups of 7 blocks -> one store DMA) ----
    psT2 = tc.alloc_tile_pool(name="psT2", bufs=2, space="PSUM")
    for g0 in range(0, NBLK, 7):
        psY = psT2.tile([128, 1792], F32, tag="psY", name="psY")
        for j in range(7):
            b = g0 + j
            for m in range(2):
                nc.tensor.matmul(psY[:, 256 * j + 128 * m : 256 * j + 128 * m + 128].bitcast(MMDT),
                                 x2[m][:, 128 * b : 128 * b + 128].bitcast(MMDT), idr(),
                                 is_transpose=True, start=True, stop=True)
        yq = pffn.tile([128, 1792], F32, tag="yq", name="yq")
        ACT.activation(yq[:], psY[:], AF.Copy)
        nc.sync.dma_start(AP(y_own, g0 * 128 * 256, [[256, 128], [128 * 256, 7], [1, 256]]), yq[:])
    psT2.release()
    pffn.release()
    p_x1.release()
    p_fw.release()
    p_out.release()
    p_own.release()
    p_pk.release()
    st.release()


def _lnorm(tc, nc, pool, lq, qc, dst, gemm_k, gemm_w, gemm_b, resid, g, bb,
           onesc, eone1, inplace=None, dst_dt=None):
    """dst[m] = LN(x)*g + b over channels (2 x 128 partitions).

    x = resid + (W.T @ gemm_k + gemm_b) when gemm_w given, else x = inplace.
    """
    ACT, DVE = nc.scalar, nc.vector
    psl = tc.alloc_tile_pool(name="psln", bufs=1, space="PSUM")
    eps = pool.tile([1, 1], F32, tag="ln_eps", name="ln_eps")
    nc.any.memset(eps[:], 1e-5)
    for (o, n) in _nchunks(lq, qc):
        xs = []
        if gemm_w is not None:
            for m in range(2):
                ps = psl.tile([128, 448], F32, tag=f"ln_ps{m}", name=f"ln_ps{m}")
                for k in range(2):
                    nc.tensor.matmul(ps[:, :n], gemm_w[k][m].bitcast(MMDT), gemm_k[k][:, o : o + n].bitcast(MMDT), start=(k == 0), stop=(k == 1))
                x = pool.tile([128, 448], F32, tag=f"ln_x{m}", name=f"ln_x{m}")
                ACT.activation(x[:, :n].bitcast(MMDT), ps[:, :n], AF.Identity, bias=gemm_b[m][:])
                DVE.tensor_tensor(x[:, :n].bitcast(MMDT), x[:, :n], resid[m][:, o : o + n], OP.add)
                xs.append(x[:, :n])
        else:
            xs = [inplace[m][:, o : o + n] for m in range(2)]

        ps_s = psl.tile([1, 448], F32, tag="ln_s1", name="ln_s1")
        ps_s2 = psl.tile([1, 448], F32, tag="ln_s2", name="ln_s2")
        sq = pool.tile([128, 448], F32, tag="ln_sq", name="ln_sq")
        for m in range(2):
            nc.tensor.matmul(ps_s[:, :n], onesc[:].bitcast(MMDT), xs[m].bitcast(MMDT), start=(m == 0), stop=(m == 1))
        for m in range(2):
            ACT.activation(sq[:, :n].bitcast(MMDT), xs[m], AF.Square)
            nc.tensor.matmul(ps_s2[:, :n], onesc[:].bitcast(MMDT), sq[:, :n].bitcast(MMDT), start=(m == 0), stop=(m == 1))
        mean = pool.tile([1, 448], F32, tag="ln_mean", name="ln_mean")
        ACT.activation(mean[:, :n].bitcast(MMDT), ps_s[:, :n], AF.Copy, scale=1.0 / 256.0)
        ms = pool.tile([1, 448], F32, tag="ln_ms", name="ln_ms")
        DVE.tensor_tensor(ms[:, :n], mean[:, :n], mean[:, :n], OP.mult)
        var = pool.tile([1, 448], F32, tag="ln_var", name="ln_var")
        ACT.activation(var[:, :n], ps_s2[:, :n], AF.Copy, scale=1.0 / 256.0)
        DVE.tensor_tensor(var[:, :n], var[:, :n], ms[:, :n], OP.subtract)
        std = pool.tile([1, 448], F32, tag="ln_std", name="ln_std")
        ACT.activation(std[:, :n], var[:, :n], AF.Sqrt, bias=eps[:])
        rstd = pool.tile([1, 448], F32, tag="ln_rstd", name="ln_rstd")
        DVE.reciprocal(rstd[:, :n].bitcast(MMDT), std[:, :n])
        # LN1 (with gemm) gets dedicated bcast banks so chunk i+1's gemm can
        # start before chunk i finishes; LN2 reuses the gemm tags (ps6 holds
        # 4 banks concurrently, 8 total).
        bt0, bt1 = ("ln_bm", "ln_br") if gemm_w is not None else ("ln_ps0", "ln_ps1")
        ps_bm = psl.tile([128, 448], F32, tag=bt0, name="ln_bm")
        ps_br = psl.tile([128, 448], F32, tag=bt1, name="ln_br")
        nc.tensor.matmul(ps_bm[:, :n], eone1[:].bitcast(MMDT), mean[:, :n].bitcast(MMDT), start=True, stop=True)
        nc.tensor.matmul(ps_br[:, :n], eone1[:].bitcast(MMDT), rstd[:, :n].bitcast(MMDT), start=True, stop=True)
        for m in range(2):
            t = pool.tile([128, 448], F32, tag="ln_t", name="ln_t")
            DVE.tensor_tensor(t[:, :n], xs[m], ps_bm[:, :n], OP.subtract)
            DVE.tensor_tensor(t[:, :n], t[:, :n], ps_br[:, :n], OP.mult)
            ACT.activation(dst[m][:, o : o + n].bitcast(dst_dt or MMDT), t[:, :n], AF.Identity, bias=bb[m][:], scale=g[m][:])
    psl.release()


# ======================= host side =======================

_CACHE = {}


def _get_program():
    if "nc" not in _CACHE:
        nc, names, consts = build_program()
        _CACHE["nc"] = nc
        _CACHE["names"] = names
        _CACHE["consts"] = consts
    return _CACHE["nc"], _CACHE["names"], _CACHE["consts"]


def make_in_maps(inputs, lq=LQ, consts=None):
    consts = consts if consts is not None else _host_consts()
    src = np.ascontiguousarray(np.asarray(inputs["src"], np.float32))
    pos = np.ascontiguousarray(np.asarray(inputs["pos"], np.float32))
    ref = np.ascontiguousarray(np.asarray(inputs["reference_points"], np.float32))
    weights = {
        k: np.ascontiguousarray(np.asarray(inputs[k]), np.float32)
        for k in ["W_val", "b_val", "W_off", "b_off", "W_attn", "b_attn",
                   "W_out", "b_out", "ln1_g", "ln1_b", "lin1_W", "lin1_b",
                   "lin2_W", "lin2_b", "ln2_g", "ln2_b"]
    }
    # permute the 96 (h,l,p) rows into (l,p,h) order
    perm = _perm96()
    perm2 = np.stack([2 * perm, 2 * perm + 1], axis=1).reshape(-1)
    weights["W_off"] = np.ascontiguousarray(weights["W_off"][:, perm2])
    weights["b_off"] = np.ascontiguousarray(weights["b_off"][perm2])
    weights["W_attn"] = np.ascontiguousarray(weights["W_attn"][:, perm])
    weights["b_attn"] = np.ascontiguousarray(weights["b_attn"][perm])
    in_maps = []
    for core in range(8):
        b, half = core // 2, core % 2
        o = half * lq
        m = {
            "src_full": src[b],
            "src_own": np.ascontiguousarray(src[b, o : o + lq]),
            "pos_own": np.ascontiguousarray(pos[b, o : o + lq]),
            "ref_own": np.ascontiguousarray(ref[b, o : o + lq]),
        }
        m.update(weights)
        m.update(consts)
        in_maps.append(m)
    return in_maps


def kernel(**inputs):
    nc, names, consts = _get_program()
    in_maps = make_in_maps(inputs, consts=consts)
    res = run_bass_kernel_spmd(nc, in_maps, core_ids=list(range(8)))
    out = np.zeros((B, L, C), np.float32)
    for core in range(8):
        b, half = core // 2, core % 2
        o = half * LQ
        out[b, o : o + LQ] = res.results[core]["y_own"]
    return out
